# revision 2
# baseline (speedup 1.0000x reference)
import sys
sys.path.insert(0, '/opt/trn_rl_repo')
import numpy as np

B = 16
H = 1024
W = 1024
K = 21
PAD = 10
NCORES = 8
WR = 148          # warp rows held per core (128 + 2*PAD)
HALF = 74
JCH = 32
NSTEP = 8
NGRP = 8
CPIX = HALF * JCH          # 2368 pixels per chunk
SLAB_R, SLAB_C = 48, 76
SLAB_E = SLAB_R * SLAB_C   # 3648
NI16 = CPIX // 16          # 148 idx cols per gather plane
LHW = 2 * K * 128          # 5376
NPIX = B * H * W           # 16777216
OUTN = NCORES * B * 128 * W  # flat gathered output length
CV = 1024                  # f16->f32 conversion chunk (per partition)
NCV = NPIX // (128 * CV)   # 128 chunks

LAST_EXEC_NS = None
PHASES = {}

_RT = {}


def _build_nc():
    import concourse.bacc as bacc
    import concourse.mybir as mybir
    import concourse.tile as tile
    import concourse.bass as bass
    from contextlib import ExitStack

    f32 = mybir.dt.float32
    f32r = mybir.dt.float32r
    f16 = mybir.dt.float16
    u16 = mybir.dt.uint16
    i32 = mybir.dt.int32
    sub_op = mybir.AluOpType.subtract
    mul_op = mybir.AluOpType.mult
    add_op = mybir.AluOpType.add

    import os
    dbg = bool(os.environ.get("BASSK_DEBUG"))
    nc = bacc.Bacc(num_devices=NCORES)
    src_d = nc.declare_dram_parameter("src", [NPIX // NCORES], f16, isOutput=False)
    if dbg:
        dbg_src_d = nc.declare_dram_parameter("dbg_src", [131072], f32, isOutput=True)
        dbg_slab_d = nc.declare_dram_parameter("dbg_slab", [128, SLAB_E], f32, isOutput=True)
        dbg_comb_d = nc.declare_dram_parameter("dbg_comb", [128, CPIX], f32, isOutput=True)
    soff_d = nc.declare_dram_parameter("soff", [NSTEP, 128, SLAB_R], i32, isOutput=False)
    idx_d = nc.declare_dram_parameter("idx", [NSTEP, 128, 2 * NI16], u16, isOutput=False)
    wts_d = nc.declare_dram_parameter("wts", [NSTEP, 128, 2 * CPIX], f32, isOutput=False)
    lh_d = nc.declare_dram_parameter("lh", [128, LHW], f32r, isOutput=False)
    out_d = nc.declare_dram_parameter("out", [OUTN], f16, isOutput=True)
    RG = [list(range(NCORES))]

    with ExitStack() as ctx:
        tc = ctx.enter_context(tile.TileContext(nc))
        const = ctx.enter_context(tc.tile_pool(name="const", bufs=1))
        dpool = ctx.enter_context(tc.tile_pool(name="dsc", bufs=1, space="DRAM"))
        vpool = ctx.enter_context(tc.tile_pool(name="cvt", bufs=2))
        spool = ctx.enter_context(tc.tile_pool(name="slab", bufs=2))
        ipool = ctx.enter_context(tc.tile_pool(name="idx", bufs=2))
        wpool = ctx.enter_context(tc.tile_pool(name="wts", bufs=2))
        cpool = ctx.enter_context(tc.tile_pool(name="comb", bufs=2))
        gpool = ctx.enter_context(tc.tile_pool(name="gath", bufs=2))
        tpool = ctx.enter_context(tc.tile_pool(name="tmp", bufs=2))
        rpool = ctx.enter_context(tc.tile_pool(name="rhs", bufs=2))
        opool = ctx.enter_context(tc.tile_pool(name="ot", bufs=2))
        pspool = ctx.enter_context(tc.tile_pool(name="ps", bufs=2, space="PSUM"))

        # internal DRAM
        cc_src16i = dpool.tile([NPIX // NCORES], f16)
        cc_src16 = dpool.tile([NPIX], f16, addr_space="Shared")
        srcf = dpool.tile([NPIX, 1], f32)
        cc_outi = dpool.tile([B, 128, W], f16)
        cc_out = dpool.tile([OUTN], f16, addr_space="Shared")
        scratch = dpool.tile([B, WR, W + 2 * PAD], f32r)

        # batch-sharded src (2 images per core) -> AllGather to full src
        nc.gpsimd.dma_start(cc_src16i[:], src_d[:])
        nc.gpsimd.collective_compute(
            "AllGather", mybir.AluOpType.bypass, replica_groups=RG,
            ins=[cc_src16i.opt()], outs=[cc_src16.opt()])

        # upconvert f16 -> f32 into srcf
        for k in range(NCV):
            t16 = vpool.tile([128, CV], f16)
            nc.sync.dma_start(t16[:], cc_src16[k * 128 * CV:(k + 1) * 128 * CV])
            t32 = vpool.tile([128, CV], f32)
            nc.vector.tensor_copy(t32[:], t16[:])
            nc.sync.dma_start(srcf[k * 128 * CV:(k + 1) * 128 * CV, :], t32[:])

        if dbg:
            nc.sync.dma_start(dbg_src_d[:], srcf[0:131072, :])

        lh_t = const.tile([128, LHW], f32r)
        nc.sync.dma_start(lh_t[:], lh_d[:, :])

        zt = const.tile([B, WR, PAD], f32)
        nc.vector.memset(zt[:], 0.0)
        nc.sync.dma_start(scratch[0:B, :, 0:PAD], zt[:].bitcast(f32r))
        nc.sync.dma_start(scratch[0:B, :, W + PAD:W + 2 * PAD], zt[:].bitcast(f32r))

        tt = nc.vector.tensor_tensor

        for s in range(NSTEP):
            soff_t = ipool.tile([128, SLAB_R], i32)
            nc.sync.dma_start(soff_t[:], soff_d[s, :, :])
            slab_t = spool.tile([128, SLAB_E], f32)
            for k in range(SLAB_R):
                nc.gpsimd.indirect_dma_start(
                    out=slab_t[:, k * SLAB_C:(k + 1) * SLAB_C],
                    out_offset=None,
                    in_=srcf[:, :],
                    in_offset=bass.IndirectOffsetOnAxis(ap=soff_t[:, k:k + 1], axis=0))
            slab_v = slab_t[:].rearrange('p (n d) -> p n d', d=2)
            if dbg and s == 0:
                nc.sync.dma_start(dbg_slab_d[:, :], slab_t[:])
            idx_t = ipool.tile([128, 2 * NI16], u16)
            nc.sync.dma_start(idx_t[:], idx_d[s, :, :])
            wts_t = wpool.tile([128, 2 * CPIX], f32)
            nc.sync.dma_start(wts_t[:], wts_d[s, :, :])
            comb_t = cpool.tile([128, CPIX], f32)

            for off, ln in ((0, 1024), (1024, 1024), (2048, 320)):
                G0 = gpool.tile([128, 1024, 2], f32)
                G1 = gpool.tile([128, 1024, 2], f32)
                for q in range(0, ln, 512):
                    sz = min(512, ln - q)
                    o16 = (off + q) // 16
                    nc.gpsimd.indirect_copy(
                        G0[:, q:q + sz, :], slab_v, idx_t[:, o16:o16 + sz // 16],
                        i_know_ap_gather_is_preferred=True)
                    nc.gpsimd.indirect_copy(
                        G1[:, q:q + sz, :], slab_v,
                        idx_t[:, NI16 + o16:NI16 + o16 + sz // 16],
                        i_know_ap_gather_is_preferred=True)
                d_t = tpool.tile([128, 1024], f32)
                x1_t = tpool.tile([128, 1024], f32)
                g00 = G0[:, 0:ln, 0]
                g01 = G0[:, 0:ln, 1]
                g10 = G1[:, 0:ln, 0]
                g11 = G1[:, 0:ln, 1]
                cs = comb_t[:, off:off + ln]
                wxs = wts_t[:, off:off + ln]
                wys = wts_t[:, CPIX + off:CPIX + off + ln]
                dv = d_t[:, 0:ln]
                x1 = x1_t[:, 0:ln]
                tt(dv, g01, g00, op=sub_op)
                tt(dv, dv, wxs, op=mul_op)
                tt(cs, g00, dv, op=add_op)
                tt(dv, g11, g10, op=sub_op)
                tt(dv, dv, wxs, op=mul_op)
                tt(x1, g10, dv, op=add_op)
                tt(x1, x1, cs, op=sub_op)
                tt(x1, x1, wys, op=mul_op)
                tt(cs, cs, x1, op=add_op)

            if dbg and s == 0:
                nc.sync.dma_start(dbg_comb_d[:, :], comb_t[:])
            for g in range(NGRP):
                h, jc = g // 4, 4 * s + (g % 4)
                nc.sync.dma_start(
                    scratch[0:B, HALF * h:HALF * h + HALF,
                            PAD + JCH * jc:PAD + JCH * jc + JCH],
                    comb_t[16 * g:16 * g + 16, :].bitcast(f32r))

        for img in range(B):
            for jh in range(2):
                rhs = rpool.tile([128, 2 * 532], f32r)
                nc.sync.dma_start(rhs[0:128, 0:532],
                                  scratch[img, 0:128, 512 * jh:512 * jh + 532])
                nc.sync.dma_start(rhs[0:20, 532:1064],
                                  scratch[img, 128:148, 512 * jh:512 * jh + 532])
                ps = pspool.tile([128, 512], mybir.dt.float32)
                for v in range(K):
                    nc.tensor.matmul(ps[:], lh_t[0:128, 128 * v:128 * v + 128],
                                     rhs[0:128, v:v + 512],
                                     start=(v == 0), stop=False)
                    nc.tensor.matmul(ps[:],
                                     lh_t[0:20, K * 128 + 128 * v:K * 128 + 128 * v + 128],
                                     rhs[0:20, 532 + v:532 + v + 512],
                                     start=False, stop=(v == K - 1))
                ot = opool.tile([128, 512], f16)
                nc.scalar.copy(ot[:], ps[:])
                nc.sync.dma_start(cc_outi[img, :, 512 * jh:512 * jh + 512], ot[:])

        # gather all cores' bands everywhere, write out
        nc.gpsimd.collective_compute(
            "AllGather", mybir.AluOpType.bypass, replica_groups=RG,
            ins=[cc_outi.opt()], outs=[cc_out.opt()])
        nc.gpsimd.dma_start(out_d[:], cc_out[:])

    nc.finalize()
    return nc


def _geometry(x0, y0, raw_b, raw_rc, raw_subpix):
    b = np.log1p(np.exp(np.float64(raw_b))) + 1e-8
    rc = np.log1p(np.exp(np.float64(raw_rc))) + 1e-8
    sub = 0.25 * np.tanh(np.asarray(raw_subpix, np.float64))
    xs = np.linspace(-1.0, 1.0, W)
    ys = np.linspace(-1.0, 1.0, H)
    dx = xs - np.float64(x0)
    dy = ys - np.float64(y0)
    denom = np.sqrt(dx[:, None] ** 2 + dy[None, :] ** 2 + 1e-12 + rc * rc)
    gx = xs[:, None] - b * dx[:, None] / denom + sub[0]
    gy = ys[None, :] - b * dy[None, :] / denom + sub[1]
    ix = (gx + 1.0) * 0.5 * (W - 1)
    iy = (gy + 1.0) * 0.5 * (H - 1)
    ix0 = np.floor(ix).astype(np.int64)
    iy0 = np.floor(iy).astype(np.int64)
    wx = (ix - ix0).astype(np.float32)
    wy = (iy - iy0).astype(np.float32)
    assert ix0.min() >= 0 and ix0.max() + 1 <= W - 1
    assert iy0.min() >= 0 and iy0.max() + 1 <= H - 1
    return ix0, iy0, wx, wy


def _pack_core(c, ix0, iy0, wx, wy):
    rows = np.clip(np.arange(c * 128 - PAD, c * 128 - PAD + WR), 0, H - 1)
    IX0 = ix0[rows, :]
    IY0 = iy0[rows, :]
    WX = wx[rows, :]
    WY = wy[rows, :]
    soff = np.empty((NSTEP, 128, SLAB_R), np.int32)
    idxp = np.empty((NSTEP, 128, 2 * NI16), np.uint16)
    wts = np.empty((NSTEP, 128, 2 * CPIX), np.float32)
    karr = np.arange(SLAB_R)
    for s in range(NSTEP):
        for g in range(NGRP):
            h, jc = g // 4, 4 * s + (g % 4)
            ksl = slice(HALF * h, HALF * h + HALF)
            jsl = slice(JCH * jc, JCH * jc + JCH)
            cy0 = IY0[ksl, jsl]
            cx0 = IX0[ksl, jsl]
            r0 = int(cy0.min())
            c0 = int(cx0.min())
            assert int(cy0.max()) + 1 - r0 <= SLAB_R - 1, "slab rows overflow"
            assert int(cx0.max()) + 1 - c0 <= SLAB_C - 1, "slab cols overflow"
            r0 = min(r0, H - SLAB_R)
            c0 = min(c0, W - SLAB_C)
            # per (image, slab row) flat element offsets into [B,H,W] f32 src
            for img in range(B):
                soff[s, 16 * g + img, :] = img * H * W + (r0 + karr) * W + c0
            fl0 = ((cy0 - r0) * SLAB_C + (cx0 - c0)).reshape(CPIX)
            idxp[s, 16 * g:16 * g + 16, 0:NI16] = \
                fl0.reshape(NI16, 16).T.astype(np.uint16)
            idxp[s, 16 * g:16 * g + 16, NI16:] = \
                (fl0 + SLAB_C).reshape(NI16, 16).T.astype(np.uint16)
            wts[s, 16 * g:16 * g + 16, 0:CPIX] = WX[ksl, jsl].reshape(1, CPIX)
            wts[s, 16 * g:16 * g + 16, CPIX:] = WY[ksl, jsl].reshape(1, CPIX)
    return soff, idxp, wts


def _pack_lh(c, psf):
    lh = np.zeros((128, LHW), np.float32)
    livek = (c * 128 - PAD + np.arange(128) >= 0) & (c * 128 - PAD + np.arange(128) < H)
    livek2 = (c * 128 + 118 + np.arange(20) >= 0) & (c * 128 + 118 + np.arange(20) < H)
    for v in range(K):
        for u in range(K):
            p = float(psf[u, v])
            ks = np.arange(u, 128)
            ms = np.arange(0, 128 - u)
            lh[ks, v * 128 + ms] = np.where(livek[ks], p, 0.0)
            ks2 = np.arange(0, 20)
            sel = ks2 + 1 <= u
            ks2 = ks2[sel]
            if ks2.size:
                ms2 = ks2 + 128 - u
                lh[ks2, K * 128 + v * 128 + ms2] = np.where(livek2[ks2], p, 0.0)
    return lh


def _ensure_runtime():
    if 'fn' in _RT:
        return
    import time
    import jax
    import jax.numpy as jnp
    from jax.sharding import Mesh, PartitionSpec, NamedSharding
    from jax.experimental.shard_map import shard_map
    import concourse.mybir as mybir
    from concourse import bass2jax

    t0 = time.perf_counter()
    nc = _build_nc()
    PHASES['build_nc'] = time.perf_counter() - t0

    bass2jax.install_neuronx_cc_hook()

    partition_name = (nc.partition_id_tensor.name
                      if nc.partition_id_tensor is not None else None)
    in_names, out_names, out_avals, zero_shapes = [], [], [], []
    for alloc in nc.m.functions[0].allocations:
        if not isinstance(alloc, mybir.MemoryLocationSet):
            continue
        name = alloc.memorylocations[0].name
        if alloc.kind == "ExternalInput":
            if name != partition_name:
                in_names.append(name)
        elif alloc.kind == "ExternalOutput":
            shape = tuple(alloc.tensor_shape)
            dtype = mybir.dt.np(alloc.dtype)
            out_names.append(name)
            out_avals.append(jax.core.ShapedArray(shape, dtype))
            zero_shapes.append((shape, dtype))
    n_params = len(in_names)
    all_names = in_names + out_names

    devs = jax.devices()[:NCORES]
    mesh = Mesh(np.asarray(devs), ("core",))
    P = PartitionSpec
    nsh = NamedSharding(mesh, P("core"))

    def _body(*args):
        operands = list(args)
        if partition_name is not None:
            operands.append(bass2jax.partition_id_tensor())
        outs = bass2jax._bass_exec_p.bind(
            *operands,
            out_avals=tuple(out_avals),
            in_names=tuple(all_names + ([partition_name] if partition_name else [])),
            out_names=tuple(out_names),
            lowering_input_output_aliases=(),
            sim_require_finite=False,
            sim_require_nnan=False,
            nc=nc,
        )
        return tuple(outs)

    nin = n_params + len(out_names)
    fn = jax.jit(
        shard_map(_body, mesh=mesh,
                  in_specs=(P("core"),) * nin,
                  out_specs=(P("core"),) * len(out_names),
                  check_rep=False),
        donate_argnums=tuple(range(n_params, nin)),
        keep_unused=True,
    )

    # jitted creators for the donated zero output operands (device-side, cheap)
    zout_fns = []
    for shape, dtype in zero_shapes:
        gshape = (NCORES * shape[0],) + tuple(shape[1:])
        zf = jax.jit(lambda s=gshape, d=dtype: jnp.zeros(s, d), out_shardings=nsh)
        zout_fns.append(zf)
    _RT.update(nc=nc, fn=fn, mesh=mesh, devs=devs, nsh=nsh,
               in_names=in_names, out_names=out_names,
               zout_fns=zout_fns, jax=jax, np_mod=np)


def _ensure_geometry(x0, y0, raw_b, raw_rc, raw_subpix, raw_psf):
    import time
    key = (float(x0), float(y0), float(raw_b), float(raw_rc),
           np.asarray(raw_subpix, np.float64).tobytes(),
           np.asarray(raw_psf, np.float64).tobytes())
    if _RT.get('geom_key') == key:
        return
    t0 = time.perf_counter()
    jax = _RT['jax']
    ix0, iy0, wx, wy = _geometry(float(x0), float(y0), float(raw_b),
                                 float(raw_rc), np.asarray(raw_subpix))
    psf = np.maximum(np.asarray(raw_psf, np.float64).reshape(K, K), 0.0)
    psf = psf / max(psf.sum(), 1e-12)
    psf = psf.astype(np.float32)

    soffs, idxs, wtss, lhs = [], [], [], []
    for c in range(NCORES):
        soff, idxp, wts = _pack_core(c, ix0, iy0, wx, wy)
        soffs.append(soff)
        idxs.append(idxp)
        wtss.append(wts)
        lhs.append(_pack_lh(c, psf))
    PHASES['geom_pack'] = time.perf_counter() - t0
    t0 = time.perf_counter()
    nsh = _RT['nsh']
    _RT['soff_g'] = jax.device_put(np.concatenate(soffs, axis=0), nsh)
    _RT['idx_g'] = jax.device_put(np.concatenate(idxs, axis=0), nsh)
    _RT['wts_g'] = jax.device_put(np.concatenate(wtss, axis=0), nsh)
    _RT['lh_g'] = jax.device_put(np.concatenate(lhs, axis=0), nsh)
    for a in (_RT['soff_g'], _RT['idx_g'], _RT['wts_g'], _RT['lh_g']):
        a.block_until_ready()
    PHASES['geom_upload'] = time.perf_counter() - t0
    _RT['geom_key'] = key


def kernel(src, raw_psf, x0, y0, raw_b, raw_rc, raw_subpix):
    global LAST_EXEC_NS
    import time
    t_all0 = time.perf_counter()
    _ensure_runtime()
    _ensure_geometry(x0, y0, raw_b, raw_rc, raw_subpix, raw_psf)
    jax = _RT['jax']
    devs = _RT['devs']
    mesh = _RT['mesh']
    from jax.sharding import PartitionSpec, NamedSharding
    nsh = _RT['nsh']

    t0 = time.perf_counter()
    zouts = [zf() for zf in _RT['zout_fns']]  # async, device-side
    src16 = np.asarray(src, np.float32).reshape(NPIX).astype(np.float16)
    PHASES['convert_in'] = time.perf_counter() - t0

    t0 = time.perf_counter()
    src_g = jax.device_put(src16, nsh)
    src_g.block_until_ready()
    PHASES['upload'] = time.perf_counter() - t0

    args = {
        'src': src_g,
        'soff': _RT['soff_g'],
        'idx': _RT['idx_g'],
        'wts': _RT['wts_g'],
        'lh': _RT['lh_g'],
    }
    ordered = [args[n] for n in _RT['in_names']] + zouts

    t0 = time.perf_counter()
    outs = _RT['fn'](*ordered)
    outs[0].block_until_ready()
    _RT['last_outs'] = outs
    PHASES['exec'] = time.perf_counter() - t0

    t0 = time.perf_counter()
    oi = _RT['out_names'].index('out')
    shard0 = None
    for sh in outs[oi].addressable_shards:
        if sh.index[0].start in (0, None):
            shard0 = sh
            break
    flat = np.asarray(shard0.data)
    PHASES['fetch'] = time.perf_counter() - t0

    t0 = time.perf_counter()
    o = flat.reshape(NCORES, B, 128, W).transpose(1, 0, 2, 3)
    out = np.ascontiguousarray(o.reshape(B, 1, NCORES * 128, W), np.float32)
    PHASES['convert_out'] = time.perf_counter() - t0

    LAST_EXEC_NS = int((time.perf_counter() - t_all0) * 1e9)
    return out


# revision 3
# speedup vs baseline: 1.0195x; 1.0195x over previous
import sys
sys.path.insert(0, '/opt/trn_rl_repo')
import numpy as np

B = 16
H = 1024
W = 1024
K = 21
PAD = 10
NCORES = 8
WR = 148          # warp rows held per core (128 + 2*PAD)
HALF = 74
JCH = 32
NSTEP = 8
NGRP = 8
CPIX = HALF * JCH          # 2368 pixels per chunk
SLAB_R, SLAB_C = 48, 76
SLAB_E = SLAB_R * SLAB_C   # 3648
NI16 = CPIX // 16          # 148 idx cols per gather plane
LHW = 2 * K * 128          # 5376

NIMG = 8                   # images per device call (B / NCALLS)
NCALLS = B // NIMG         # pipelined calls per kernel() invocation
NPIXC = NIMG * H * W       # per-call pixels (8M)
OUTN = NCORES * NIMG * 128 * W
CV = 1024                  # f16->f32 conversion chunk (per partition)
NCV = NPIXC // (128 * CV)  # conversion chunks

LAST_EXEC_NS = None
PHASES = {}

_RT = {}
_TPOOL = None


def _to_f16_threaded(srcv):
    global _TPOOL
    from concurrent.futures import ThreadPoolExecutor
    if _TPOOL is None:
        _TPOOL = ThreadPoolExecutor(8)
    out = np.empty(srcv.size, np.float16)
    n = srcv.size
    step = n // 8
    flat = srcv.reshape(-1)

    def w(i):
        out[i * step:(i + 1) * step] = flat[i * step:(i + 1) * step].astype(np.float16)
    list(_TPOOL.map(w, range(8)))
    return out


def _build_nc():
    import os
    import concourse.bacc as bacc
    import concourse.mybir as mybir
    import concourse.tile as tile
    import concourse.bass as bass
    from contextlib import ExitStack

    f32 = mybir.dt.float32
    f32r = mybir.dt.float32r
    f16 = mybir.dt.float16
    u16 = mybir.dt.uint16
    i32 = mybir.dt.int32
    sub_op = mybir.AluOpType.subtract
    mul_op = mybir.AluOpType.mult
    add_op = mybir.AluOpType.add

    nc = bacc.Bacc(num_devices=NCORES)
    src_d = nc.declare_dram_parameter("src", [NPIXC], f16, isOutput=False)
    soff_d = nc.declare_dram_parameter("soff", [NSTEP, 128, SLAB_R], i32, isOutput=False)
    idx_d = nc.declare_dram_parameter("idx", [NSTEP, 128, 2 * NI16], u16, isOutput=False)
    wts_d = nc.declare_dram_parameter("wts", [NSTEP, 128, 2 * CPIX], f16, isOutput=False)
    lh_d = nc.declare_dram_parameter("lh", [128, LHW], f32r, isOutput=False)
    out_d = nc.declare_dram_parameter("out", [OUTN], f16, isOutput=True)
    RG = [list(range(NCORES))]

    with ExitStack() as ctx:
        tc = ctx.enter_context(tile.TileContext(nc))
        const = ctx.enter_context(tc.tile_pool(name="const", bufs=1))
        dpool = ctx.enter_context(tc.tile_pool(name="dsc", bufs=1, space="DRAM"))
        vpool = ctx.enter_context(tc.tile_pool(name="cvt", bufs=2))
        spool = ctx.enter_context(tc.tile_pool(name="slab", bufs=2))
        ipool = ctx.enter_context(tc.tile_pool(name="idx", bufs=2))
        wpool = ctx.enter_context(tc.tile_pool(name="wts", bufs=2))
        cpool = ctx.enter_context(tc.tile_pool(name="comb", bufs=2))
        gpool = ctx.enter_context(tc.tile_pool(name="gath", bufs=2))
        tpool = ctx.enter_context(tc.tile_pool(name="tmp", bufs=2))
        rpool = ctx.enter_context(tc.tile_pool(name="rhs", bufs=2))
        opool = ctx.enter_context(tc.tile_pool(name="ot", bufs=2))
        pspool = ctx.enter_context(tc.tile_pool(name="ps", bufs=2, space="PSUM"))

        # internal DRAM
        cc_src16i = dpool.tile([NPIXC], f16)
        cc_src16 = dpool.tile([NCORES * NPIXC], f16, addr_space="Shared")
        srcf = dpool.tile([NPIXC, 1], f32)
        cc_outi = dpool.tile([NIMG, 128, W], f16)
        cc_out = dpool.tile([OUTN], f16, addr_space="Shared")
        scratch = dpool.tile([NIMG, WR, W + 2 * PAD], f32r)

        # src block lives on core 0; AllGather copies every core's (mostly
        # garbage) block, and only block 0 -- core 0's real data -- is read
        nc.gpsimd.dma_start(cc_src16i[:], src_d[:])
        nc.gpsimd.collective_compute(
            "AllGather", mybir.AluOpType.bypass, replica_groups=RG,
            ins=[cc_src16i.opt()], outs=[cc_src16.opt()])

        # upconvert f16 -> f32 into srcf
        for k in range(NCV):
            t16 = vpool.tile([128, CV], f16)
            nc.sync.dma_start(t16[:], cc_src16[k * 128 * CV:(k + 1) * 128 * CV])
            t32 = vpool.tile([128, CV], f32)
            nc.vector.tensor_copy(t32[:], t16[:])
            nc.sync.dma_start(srcf[k * 128 * CV:(k + 1) * 128 * CV, :], t32[:])

        lh_t = const.tile([128, LHW], f32r)
        nc.sync.dma_start(lh_t[:], lh_d[:, :])

        zt = const.tile([NIMG, WR, PAD], f32)
        nc.vector.memset(zt[:], 0.0)
        nc.sync.dma_start(scratch[0:NIMG, :, 0:PAD], zt[:].bitcast(f32r))
        nc.sync.dma_start(scratch[0:NIMG, :, W + PAD:W + 2 * PAD], zt[:].bitcast(f32r))

        tt = nc.vector.tensor_tensor

        for s in range(NSTEP):
            soff_t = ipool.tile([128, SLAB_R], i32)
            nc.sync.dma_start(soff_t[:], soff_d[s, :, :])
            slab_t = spool.tile([128, SLAB_E], f32)
            for k in range(SLAB_R):
                nc.gpsimd.indirect_dma_start(
                    out=slab_t[:, k * SLAB_C:(k + 1) * SLAB_C],
                    out_offset=None,
                    in_=srcf[:, :],
                    in_offset=bass.IndirectOffsetOnAxis(ap=soff_t[:, k:k + 1], axis=0))
            slab_v = slab_t[:].rearrange('p (n d) -> p n d', d=2)
            idx_t = ipool.tile([128, 2 * NI16], u16)
            nc.sync.dma_start(idx_t[:], idx_d[s, :, :])
            wts16_t = wpool.tile([128, 2 * CPIX], f16)
            nc.sync.dma_start(wts16_t[:], wts_d[s, :, :])
            wts_t = wpool.tile([128, 2 * CPIX], f32)
            nc.vector.tensor_copy(wts_t[:], wts16_t[:])
            comb_t = cpool.tile([128, CPIX], f32)

            for off, ln in ((0, 1024), (1024, 1024), (2048, 320)):
                G0 = gpool.tile([128, 1024, 2], f32)
                G1 = gpool.tile([128, 1024, 2], f32)
                for q in range(0, ln, 512):
                    sz = min(512, ln - q)
                    o16 = (off + q) // 16
                    nc.gpsimd.indirect_copy(
                        G0[:, q:q + sz, :], slab_v, idx_t[:, o16:o16 + sz // 16],
                        i_know_ap_gather_is_preferred=True)
                    nc.gpsimd.indirect_copy(
                        G1[:, q:q + sz, :], slab_v,
                        idx_t[:, NI16 + o16:NI16 + o16 + sz // 16],
                        i_know_ap_gather_is_preferred=True)
                d_t = tpool.tile([128, 1024], f32)
                x1_t = tpool.tile([128, 1024], f32)
                g00 = G0[:, 0:ln, 0]
                g01 = G0[:, 0:ln, 1]
                g10 = G1[:, 0:ln, 0]
                g11 = G1[:, 0:ln, 1]
                cs = comb_t[:, off:off + ln]
                wxs = wts_t[:, off:off + ln]
                wys = wts_t[:, CPIX + off:CPIX + off + ln]
                dv = d_t[:, 0:ln]
                x1 = x1_t[:, 0:ln]
                tt(dv, g01, g00, op=sub_op)
                tt(dv, dv, wxs, op=mul_op)
                tt(cs, g00, dv, op=add_op)
                tt(dv, g11, g10, op=sub_op)
                tt(dv, dv, wxs, op=mul_op)
                tt(x1, g10, dv, op=add_op)
                tt(x1, x1, cs, op=sub_op)
                tt(x1, x1, wys, op=mul_op)
                tt(cs, cs, x1, op=add_op)

            for g in range(NGRP):
                h, jc = g // 4, 4 * s + (g % 4)
                nc.sync.dma_start(
                    scratch[0:NIMG, HALF * h:HALF * h + HALF,
                            PAD + JCH * jc:PAD + JCH * jc + JCH],
                    comb_t[16 * g:16 * g + NIMG, :].bitcast(f32r))

        for img in range(NIMG):
            for jh in range(2):
                rhs = rpool.tile([128, 2 * 532], f32r)
                nc.sync.dma_start(rhs[0:128, 0:532],
                                  scratch[img, 0:128, 512 * jh:512 * jh + 532])
                nc.sync.dma_start(rhs[0:20, 532:1064],
                                  scratch[img, 128:148, 512 * jh:512 * jh + 532])
                ps = pspool.tile([128, 512], mybir.dt.float32)
                for v in range(K):
                    nc.tensor.matmul(ps[:], lh_t[0:128, 128 * v:128 * v + 128],
                                     rhs[0:128, v:v + 512],
                                     start=(v == 0), stop=False)
                    nc.tensor.matmul(ps[:],
                                     lh_t[0:20, K * 128 + 128 * v:K * 128 + 128 * v + 128],
                                     rhs[0:20, 532 + v:532 + v + 512],
                                     start=False, stop=(v == K - 1))
                ot = opool.tile([128, 512], f16)
                nc.scalar.copy(ot[:], ps[:])
                nc.sync.dma_start(cc_outi[img, :, 512 * jh:512 * jh + 512], ot[:])

        # gather all cores' bands everywhere, write out
        nc.gpsimd.collective_compute(
            "AllGather", mybir.AluOpType.bypass, replica_groups=RG,
            ins=[cc_outi.opt()], outs=[cc_out.opt()])
        nc.gpsimd.dma_start(out_d[:], cc_out[:])

    nc.finalize()
    return nc


def _geometry(x0, y0, raw_b, raw_rc, raw_subpix):
    b = np.log1p(np.exp(np.float64(raw_b))) + 1e-8
    rc = np.log1p(np.exp(np.float64(raw_rc))) + 1e-8
    sub = 0.25 * np.tanh(np.asarray(raw_subpix, np.float64))
    xs = np.linspace(-1.0, 1.0, W)
    ys = np.linspace(-1.0, 1.0, H)
    dx = xs - np.float64(x0)
    dy = ys - np.float64(y0)
    denom = np.sqrt(dx[:, None] ** 2 + dy[None, :] ** 2 + 1e-12 + rc * rc)
    gx = xs[:, None] - b * dx[:, None] / denom + sub[0]
    gy = ys[None, :] - b * dy[None, :] / denom + sub[1]
    ix = (gx + 1.0) * 0.5 * (W - 1)
    iy = (gy + 1.0) * 0.5 * (H - 1)
    ix0 = np.floor(ix).astype(np.int64)
    iy0 = np.floor(iy).astype(np.int64)
    wx = (ix - ix0).astype(np.float32)
    wy = (iy - iy0).astype(np.float32)
    assert ix0.min() >= 0 and ix0.max() + 1 <= W - 1
    assert iy0.min() >= 0 and iy0.max() + 1 <= H - 1
    return ix0, iy0, wx, wy


def _pack_core(c, ix0, iy0, wx, wy):
    rows = np.clip(np.arange(c * 128 - PAD, c * 128 - PAD + WR), 0, H - 1)
    IX0 = ix0[rows, :]
    IY0 = iy0[rows, :]
    WX = wx[rows, :]
    WY = wy[rows, :]
    soff = np.zeros((NSTEP, 128, SLAB_R), np.int32)
    idxp = np.empty((NSTEP, 128, 2 * NI16), np.uint16)
    wts = np.empty((NSTEP, 128, 2 * CPIX), np.float32)
    karr = np.arange(SLAB_R)
    for s in range(NSTEP):
        for g in range(NGRP):
            h, jc = g // 4, 4 * s + (g % 4)
            ksl = slice(HALF * h, HALF * h + HALF)
            jsl = slice(JCH * jc, JCH * jc + JCH)
            cy0 = IY0[ksl, jsl]
            cx0 = IX0[ksl, jsl]
            r0 = int(cy0.min())
            c0 = int(cx0.min())
            assert int(cy0.max()) + 1 - r0 <= SLAB_R - 1, "slab rows overflow"
            assert int(cx0.max()) + 1 - c0 <= SLAB_C - 1, "slab cols overflow"
            r0 = min(r0, H - SLAB_R)
            c0 = min(c0, W - SLAB_C)
            for img in range(NIMG):
                soff[s, 16 * g + img, :] = img * H * W + (r0 + karr) * W + c0
            fl0 = ((cy0 - r0) * SLAB_C + (cx0 - c0)).reshape(CPIX)
            idxp[s, 16 * g:16 * g + 16, 0:NI16] = \
                fl0.reshape(NI16, 16).T.astype(np.uint16)
            idxp[s, 16 * g:16 * g + 16, NI16:] = \
                (fl0 + SLAB_C).reshape(NI16, 16).T.astype(np.uint16)
            wts[s, 16 * g:16 * g + 16, 0:CPIX] = WX[ksl, jsl].reshape(1, CPIX)
            wts[s, 16 * g:16 * g + 16, CPIX:] = WY[ksl, jsl].reshape(1, CPIX)
    return soff, idxp, wts


def _pack_lh(c, psf):
    lh = np.zeros((128, LHW), np.float32)
    livek = (c * 128 - PAD + np.arange(128) >= 0) & (c * 128 - PAD + np.arange(128) < H)
    livek2 = (c * 128 + 118 + np.arange(20) >= 0) & (c * 128 + 118 + np.arange(20) < H)
    for v in range(K):
        for u in range(K):
            p = float(psf[u, v])
            ks = np.arange(u, 128)
            ms = np.arange(0, 128 - u)
            lh[ks, v * 128 + ms] = np.where(livek[ks], p, 0.0)
            ks2 = np.arange(0, 20)
            sel = ks2 + 1 <= u
            ks2 = ks2[sel]
            if ks2.size:
                ms2 = ks2 + 128 - u
                lh[ks2, K * 128 + v * 128 + ms2] = np.where(livek2[ks2], p, 0.0)
    return lh


def _ensure_runtime():
    if 'fn' in _RT:
        return
    import time
    import jax
    import jax.numpy as jnp
    from jax.sharding import Mesh, PartitionSpec, NamedSharding
    from jax.experimental.shard_map import shard_map
    import concourse.mybir as mybir
    from concourse import bass2jax

    t0 = time.perf_counter()
    nc = _build_nc()
    PHASES['build_nc'] = time.perf_counter() - t0

    bass2jax.install_neuronx_cc_hook()

    partition_name = (nc.partition_id_tensor.name
                      if nc.partition_id_tensor is not None else None)
    in_names, out_names, out_avals, zero_shapes = [], [], [], []
    for alloc in nc.m.functions[0].allocations:
        if not isinstance(alloc, mybir.MemoryLocationSet):
            continue
        name = alloc.memorylocations[0].name
        if alloc.kind == "ExternalInput":
            if name != partition_name:
                in_names.append(name)
        elif alloc.kind == "ExternalOutput":
            shape = tuple(alloc.tensor_shape)
            dtype = mybir.dt.np(alloc.dtype)
            out_names.append(name)
            out_avals.append(jax.core.ShapedArray(shape, dtype))
            zero_shapes.append((shape, dtype))
    n_params = len(in_names)
    all_names = in_names + out_names

    devs = jax.devices()[:NCORES]
    mesh = Mesh(np.asarray(devs), ("core",))
    P = PartitionSpec
    nsh = NamedSharding(mesh, P("core"))

    def _body(*args):
        operands = list(args)
        if partition_name is not None:
            operands.append(bass2jax.partition_id_tensor())
        outs = bass2jax._bass_exec_p.bind(
            *operands,
            out_avals=tuple(out_avals),
            in_names=tuple(all_names + ([partition_name] if partition_name else [])),
            out_names=tuple(out_names),
            lowering_input_output_aliases=(),
            sim_require_finite=False,
            sim_require_nnan=False,
            nc=nc,
        )
        return tuple(outs)

    nin = n_params + len(out_names)
    fn = jax.jit(
        shard_map(_body, mesh=mesh,
                  in_specs=(P("core"),) * nin,
                  out_specs=(P("core"),) * len(out_names),
                  check_rep=False),
        donate_argnums=tuple(range(n_params, nin)),
        keep_unused=True,
    )

    zout_fns = []
    for shape, dtype in zero_shapes:
        gshape = (NCORES * shape[0],) + tuple(shape[1:])
        zf = jax.jit(lambda s=gshape, d=dtype: jnp.zeros(s, d), out_shardings=nsh)
        zout_fns.append(zf)

    # persistent dummy src shards for cores 1..7 (content never read)
    dummies = []
    for i in range(1, NCORES):
        df = jax.jit(lambda: jnp.zeros((NPIXC,), jnp.float16),
                     out_shardings=jax.sharding.SingleDeviceSharding(devs[i]))
        dummies.append(df())
    for a in dummies:
        a.block_until_ready()

    _RT.update(nc=nc, fn=fn, mesh=mesh, devs=devs, nsh=nsh,
               in_names=in_names, out_names=out_names,
               zout_fns=zout_fns, dummies=dummies, donors=[], jax=jax, np_mod=np)


def _ensure_geometry(x0, y0, raw_b, raw_rc, raw_subpix, raw_psf):
    import time
    key = (float(x0), float(y0), float(raw_b), float(raw_rc),
           np.asarray(raw_subpix, np.float64).tobytes(),
           np.asarray(raw_psf, np.float64).tobytes())
    if _RT.get('geom_key') == key:
        return
    t0 = time.perf_counter()
    jax = _RT['jax']
    ix0, iy0, wx, wy = _geometry(float(x0), float(y0), float(raw_b),
                                 float(raw_rc), np.asarray(raw_subpix))
    psf = np.maximum(np.asarray(raw_psf, np.float64).reshape(K, K), 0.0)
    psf = psf / max(psf.sum(), 1e-12)
    psf = psf.astype(np.float32)

    soffs, idxs, wtss, lhs = [], [], [], []
    for c in range(NCORES):
        soff, idxp, wts = _pack_core(c, ix0, iy0, wx, wy)
        soffs.append(soff)
        idxs.append(idxp)
        wtss.append(wts)
        lhs.append(_pack_lh(c, psf))
    PHASES['geom_pack'] = time.perf_counter() - t0
    t0 = time.perf_counter()
    nsh = _RT['nsh']
    _RT['soff_g'] = jax.device_put(np.concatenate(soffs, axis=0), nsh)
    _RT['idx_g'] = jax.device_put(np.concatenate(idxs, axis=0), nsh)
    _RT['wts_g'] = jax.device_put(np.concatenate(wtss, axis=0).astype(np.float16), nsh)
    _RT['lh_g'] = jax.device_put(np.concatenate(lhs, axis=0), nsh)
    for a in (_RT['soff_g'], _RT['idx_g'], _RT['wts_g'], _RT['lh_g']):
        a.block_until_ready()
    PHASES['geom_upload'] = time.perf_counter() - t0
    _RT['geom_key'] = key


def _fetch_shard0(arr):
    for sh in arr.addressable_shards:
        if sh.index[0].start in (0, None):
            return np.asarray(sh.data)
    raise RuntimeError("shard0 not found")


def kernel(src, raw_psf, x0, y0, raw_b, raw_rc, raw_subpix):
    global LAST_EXEC_NS
    import time
    t_all0 = time.perf_counter()
    _ensure_runtime()
    _ensure_geometry(x0, y0, raw_b, raw_rc, raw_subpix, raw_psf)
    jax = _RT['jax']
    nsh = _RT['nsh']
    fn = _RT['fn']
    geom = [_RT['soff_g'], _RT['idx_g'], _RT['wts_g'], _RT['lh_g']]
    gmap = dict(zip(['soff', 'idx', 'wts', 'lh'], geom))

    srcv = np.asarray(src, np.float32).reshape(NCALLS, NPIXC)
    PHASES['convert_in'] = 0.0

    # pipelined calls: enqueue everything, then fetch in order (duplex relay
    # overlaps call k+1 upload/exec with call k download)
    t0 = time.perf_counter()
    donors = _RT['donors']
    _RT['donors'] = []
    outs_per_call = []
    oi = _RT['out_names'].index('out')
    for k in range(NCALLS):
        if donors:
            zouts = [donors.pop()]
        else:
            zouts = [zf() for zf in _RT['zout_fns']]
        s0 = jax.device_put(srcv[k].astype(np.float16), _RT['devs'][0])
        src_g = jax.make_array_from_single_device_arrays(
            (NCORES * NPIXC,), nsh, [s0] + _RT['dummies'])
        args = {'src': src_g, **gmap}
        ordered = [args[n] for n in _RT['in_names']] + zouts
        outs_per_call.append(fn(*ordered))
    PHASES['dispatch'] = time.perf_counter() - t0

    t0 = time.perf_counter()
    out = np.empty((B, 1, H, W), np.float32)
    for k, outs in enumerate(outs_per_call):
        flat = _fetch_shard0(outs[oi])
        # convert while the next call's download is still in flight
        o = flat.reshape(NCORES, NIMG, 128, W).astype(np.float32)
        out[k * NIMG:(k + 1) * NIMG, 0] = \
            o.transpose(1, 0, 2, 3).reshape(NIMG, NCORES * 128, W)
    _RT['donors'] = [outs[oi] for outs in outs_per_call]
    PHASES['exec_fetch'] = time.perf_counter() - t0
    PHASES['convert_out'] = 0.0

    del outs_per_call
    import gc
    gc.collect()
    LAST_EXEC_NS = int((time.perf_counter() - t_all0) * 1e9)
    return out


# revision 4
# speedup vs baseline: 1.1946x; 1.1718x over previous
import sys
sys.path.insert(0, '/opt/trn_rl_repo')
import numpy as np

B = 16
H = 1024
W = 1024
K = 21
PAD = 10
NCORES = 8
WR = 148          # warp rows held per core (128 + 2*PAD)
HALF = 74
JCH = 32
NSTEP = 8
NGRP = 8
CPIX = HALF * JCH          # 2368 pixels per chunk
SLAB_R, SLAB_C = 48, 76
SLAB_E = SLAB_R * SLAB_C   # 3648
NI16 = CPIX // 16          # 148 idx cols per gather plane
LHW = 2 * K * 128          # 5376

NIMG = 8                   # images per device call (B / NCALLS)
NCALLS = B // NIMG         # pipelined calls per kernel() invocation
NPIXC = NIMG * H * W       # per-call pixels (8M)
OUTN = NCORES * NIMG * 128 * W
CV = 1024                  # f16->f32 conversion chunk (per partition)
NCV = NPIXC // (128 * CV)  # conversion chunks

LAST_EXEC_NS = None
PHASES = {}

_RT = {}
_TPOOL = None


def _to_f16_threaded(srcv):
    global _TPOOL
    from concurrent.futures import ThreadPoolExecutor
    if _TPOOL is None:
        _TPOOL = ThreadPoolExecutor(8)
    out = np.empty(srcv.size, np.float16)
    n = srcv.size
    step = n // 8
    flat = srcv.reshape(-1)

    def w(i):
        out[i * step:(i + 1) * step] = flat[i * step:(i + 1) * step].astype(np.float16)
    list(_TPOOL.map(w, range(8)))
    return out


def _build_nc():
    import os
    import concourse.bacc as bacc
    import concourse.mybir as mybir
    import concourse.tile as tile
    import concourse.bass as bass
    from contextlib import ExitStack

    f32 = mybir.dt.float32
    f32r = mybir.dt.float32r
    f16 = mybir.dt.float16
    u16 = mybir.dt.uint16
    i32 = mybir.dt.int32
    sub_op = mybir.AluOpType.subtract
    mul_op = mybir.AluOpType.mult
    add_op = mybir.AluOpType.add

    nc = bacc.Bacc(num_devices=NCORES)
    src_d = nc.declare_dram_parameter("src", [NPIXC], f16, isOutput=False)
    soff_d = nc.declare_dram_parameter("soff", [NSTEP, 128, SLAB_R], i32, isOutput=False)
    idx_d = nc.declare_dram_parameter("idx", [NSTEP, 128, 2 * NI16], u16, isOutput=False)
    wts_d = nc.declare_dram_parameter("wts", [NSTEP, 128, 2 * CPIX], f16, isOutput=False)
    lh_d = nc.declare_dram_parameter("lh", [128, LHW], f32r, isOutput=False)
    out_d = nc.declare_dram_parameter("out", [OUTN], f16, isOutput=True)
    RG = [list(range(NCORES))]

    with ExitStack() as ctx:
        tc = ctx.enter_context(tile.TileContext(nc))
        const = ctx.enter_context(tc.tile_pool(name="const", bufs=1))
        dpool = ctx.enter_context(tc.tile_pool(name="dsc", bufs=1, space="DRAM"))
        vpool = ctx.enter_context(tc.tile_pool(name="cvt", bufs=2))
        spool = ctx.enter_context(tc.tile_pool(name="slab", bufs=2))
        ipool = ctx.enter_context(tc.tile_pool(name="idx", bufs=2))
        wpool = ctx.enter_context(tc.tile_pool(name="wts", bufs=2))
        cpool = ctx.enter_context(tc.tile_pool(name="comb", bufs=2))
        gpool = ctx.enter_context(tc.tile_pool(name="gath", bufs=2))
        tpool = ctx.enter_context(tc.tile_pool(name="tmp", bufs=2))
        rpool = ctx.enter_context(tc.tile_pool(name="rhs", bufs=2))
        opool = ctx.enter_context(tc.tile_pool(name="ot", bufs=2))
        pspool = ctx.enter_context(tc.tile_pool(name="ps", bufs=2, space="PSUM"))

        # internal DRAM
        cc_src16i = dpool.tile([NPIXC], f16)
        cc_src16 = dpool.tile([NCORES * NPIXC], f16, addr_space="Shared")
        srcf = dpool.tile([NPIXC, 1], f32)
        cc_outi = dpool.tile([NIMG, 128, W], f16)
        cc_out = dpool.tile([OUTN], f16, addr_space="Shared")
        scratch = dpool.tile([NIMG, WR, W + 2 * PAD], f32r)

        # src block lives on core 0; AllGather copies every core's (mostly
        # garbage) block, and only block 0 -- core 0's real data -- is read
        nc.gpsimd.dma_start(cc_src16i[:], src_d[:])
        nc.gpsimd.collective_compute(
            "AllGather", mybir.AluOpType.bypass, replica_groups=RG,
            ins=[cc_src16i.opt()], outs=[cc_src16.opt()])

        # upconvert f16 -> f32 into srcf
        for k in range(NCV):
            t16 = vpool.tile([128, CV], f16)
            nc.sync.dma_start(t16[:], cc_src16[k * 128 * CV:(k + 1) * 128 * CV])
            t32 = vpool.tile([128, CV], f32)
            nc.vector.tensor_copy(t32[:], t16[:])
            nc.sync.dma_start(srcf[k * 128 * CV:(k + 1) * 128 * CV, :], t32[:])

        lh_t = const.tile([128, LHW], f32r)
        nc.sync.dma_start(lh_t[:], lh_d[:, :])

        zt = const.tile([NIMG, WR, PAD], f32)
        nc.vector.memset(zt[:], 0.0)
        nc.sync.dma_start(scratch[0:NIMG, :, 0:PAD], zt[:].bitcast(f32r))
        nc.sync.dma_start(scratch[0:NIMG, :, W + PAD:W + 2 * PAD], zt[:].bitcast(f32r))

        tt = nc.vector.tensor_tensor

        for s in range(NSTEP):
            soff_t = ipool.tile([128, SLAB_R], i32)
            nc.sync.dma_start(soff_t[:], soff_d[s, :, :])
            slab_t = spool.tile([128, SLAB_E], f32)
            for k in range(SLAB_R):
                nc.gpsimd.indirect_dma_start(
                    out=slab_t[:, k * SLAB_C:(k + 1) * SLAB_C],
                    out_offset=None,
                    in_=srcf[:, :],
                    in_offset=bass.IndirectOffsetOnAxis(ap=soff_t[:, k:k + 1], axis=0))
            slab_v = slab_t[:].rearrange('p (n d) -> p n d', d=2)
            idx_t = ipool.tile([128, 2 * NI16], u16)
            nc.sync.dma_start(idx_t[:], idx_d[s, :, :])
            wts16_t = wpool.tile([128, 2 * CPIX], f16)
            nc.sync.dma_start(wts16_t[:], wts_d[s, :, :])
            wts_t = wpool.tile([128, 2 * CPIX], f32)
            nc.vector.tensor_copy(wts_t[:], wts16_t[:])
            comb_t = cpool.tile([128, CPIX], f32)

            for off, ln in ((0, 1024), (1024, 1024), (2048, 320)):
                G0 = gpool.tile([128, 1024, 2], f32)
                G1 = gpool.tile([128, 1024, 2], f32)
                for q in range(0, ln, 512):
                    sz = min(512, ln - q)
                    o16 = (off + q) // 16
                    nc.gpsimd.indirect_copy(
                        G0[:, q:q + sz, :], slab_v, idx_t[:, o16:o16 + sz // 16],
                        i_know_ap_gather_is_preferred=True)
                    nc.gpsimd.indirect_copy(
                        G1[:, q:q + sz, :], slab_v,
                        idx_t[:, NI16 + o16:NI16 + o16 + sz // 16],
                        i_know_ap_gather_is_preferred=True)
                d_t = tpool.tile([128, 1024], f32)
                x1_t = tpool.tile([128, 1024], f32)
                g00 = G0[:, 0:ln, 0]
                g01 = G0[:, 0:ln, 1]
                g10 = G1[:, 0:ln, 0]
                g11 = G1[:, 0:ln, 1]
                cs = comb_t[:, off:off + ln]
                wxs = wts_t[:, off:off + ln]
                wys = wts_t[:, CPIX + off:CPIX + off + ln]
                dv = d_t[:, 0:ln]
                x1 = x1_t[:, 0:ln]
                tt(dv, g01, g00, op=sub_op)
                tt(dv, dv, wxs, op=mul_op)
                tt(cs, g00, dv, op=add_op)
                tt(dv, g11, g10, op=sub_op)
                tt(dv, dv, wxs, op=mul_op)
                tt(x1, g10, dv, op=add_op)
                tt(x1, x1, cs, op=sub_op)
                tt(x1, x1, wys, op=mul_op)
                tt(cs, cs, x1, op=add_op)

            for g in range(NGRP):
                h, jc = g // 4, 4 * s + (g % 4)
                nc.sync.dma_start(
                    scratch[0:NIMG, HALF * h:HALF * h + HALF,
                            PAD + JCH * jc:PAD + JCH * jc + JCH],
                    comb_t[16 * g:16 * g + NIMG, :].bitcast(f32r))

        for img in range(NIMG):
            for jh in range(2):
                rhs = rpool.tile([128, 2 * 532], f32r)
                nc.sync.dma_start(rhs[0:128, 0:532],
                                  scratch[img, 0:128, 512 * jh:512 * jh + 532])
                nc.sync.dma_start(rhs[0:20, 532:1064],
                                  scratch[img, 128:148, 512 * jh:512 * jh + 532])
                ps = pspool.tile([128, 512], mybir.dt.float32)
                for v in range(K):
                    nc.tensor.matmul(ps[:], lh_t[0:128, 128 * v:128 * v + 128],
                                     rhs[0:128, v:v + 512],
                                     start=(v == 0), stop=False)
                    nc.tensor.matmul(ps[:],
                                     lh_t[0:20, K * 128 + 128 * v:K * 128 + 128 * v + 128],
                                     rhs[0:20, 532 + v:532 + v + 512],
                                     start=False, stop=(v == K - 1))
                ot = opool.tile([128, 512], f16)
                nc.scalar.copy(ot[:], ps[:])
                nc.sync.dma_start(cc_outi[img, :, 512 * jh:512 * jh + 512], ot[:])

        # gather all cores' bands everywhere, write out
        nc.gpsimd.collective_compute(
            "AllGather", mybir.AluOpType.bypass, replica_groups=RG,
            ins=[cc_outi.opt()], outs=[cc_out.opt()])
        nc.gpsimd.dma_start(out_d[:], cc_out[:])

    nc.finalize()
    return nc


def _geometry(x0, y0, raw_b, raw_rc, raw_subpix):
    b = np.log1p(np.exp(np.float64(raw_b))) + 1e-8
    rc = np.log1p(np.exp(np.float64(raw_rc))) + 1e-8
    sub = 0.25 * np.tanh(np.asarray(raw_subpix, np.float64))
    xs = np.linspace(-1.0, 1.0, W)
    ys = np.linspace(-1.0, 1.0, H)
    dx = xs - np.float64(x0)
    dy = ys - np.float64(y0)
    denom = np.sqrt(dx[:, None] ** 2 + dy[None, :] ** 2 + 1e-12 + rc * rc)
    gx = xs[:, None] - b * dx[:, None] / denom + sub[0]
    gy = ys[None, :] - b * dy[None, :] / denom + sub[1]
    ix = (gx + 1.0) * 0.5 * (W - 1)
    iy = (gy + 1.0) * 0.5 * (H - 1)
    ix0 = np.floor(ix).astype(np.int64)
    iy0 = np.floor(iy).astype(np.int64)
    wx = (ix - ix0).astype(np.float32)
    wy = (iy - iy0).astype(np.float32)
    assert ix0.min() >= 0 and ix0.max() + 1 <= W - 1
    assert iy0.min() >= 0 and iy0.max() + 1 <= H - 1
    return ix0, iy0, wx, wy


def _pack_core(c, ix0, iy0, wx, wy):
    rows = np.clip(np.arange(c * 128 - PAD, c * 128 - PAD + WR), 0, H - 1)
    IX0 = ix0[rows, :]
    IY0 = iy0[rows, :]
    WX = wx[rows, :]
    WY = wy[rows, :]
    soff = np.zeros((NSTEP, 128, SLAB_R), np.int32)
    idxp = np.empty((NSTEP, 128, 2 * NI16), np.uint16)
    wts = np.empty((NSTEP, 128, 2 * CPIX), np.float32)
    karr = np.arange(SLAB_R)
    for s in range(NSTEP):
        for g in range(NGRP):
            h, jc = g // 4, 4 * s + (g % 4)
            ksl = slice(HALF * h, HALF * h + HALF)
            jsl = slice(JCH * jc, JCH * jc + JCH)
            cy0 = IY0[ksl, jsl]
            cx0 = IX0[ksl, jsl]
            r0 = int(cy0.min())
            c0 = int(cx0.min())
            assert int(cy0.max()) + 1 - r0 <= SLAB_R - 1, "slab rows overflow"
            assert int(cx0.max()) + 1 - c0 <= SLAB_C - 1, "slab cols overflow"
            r0 = min(r0, H - SLAB_R)
            c0 = min(c0, W - SLAB_C)
            for img in range(NIMG):
                soff[s, 16 * g + img, :] = img * H * W + (r0 + karr) * W + c0
            fl0 = ((cy0 - r0) * SLAB_C + (cx0 - c0)).reshape(CPIX)
            idxp[s, 16 * g:16 * g + 16, 0:NI16] = \
                fl0.reshape(NI16, 16).T.astype(np.uint16)
            idxp[s, 16 * g:16 * g + 16, NI16:] = \
                (fl0 + SLAB_C).reshape(NI16, 16).T.astype(np.uint16)
            wts[s, 16 * g:16 * g + 16, 0:CPIX] = WX[ksl, jsl].reshape(1, CPIX)
            wts[s, 16 * g:16 * g + 16, CPIX:] = WY[ksl, jsl].reshape(1, CPIX)
    return soff, idxp, wts


def _pack_lh(c, psf):
    lh = np.zeros((128, LHW), np.float32)
    livek = (c * 128 - PAD + np.arange(128) >= 0) & (c * 128 - PAD + np.arange(128) < H)
    livek2 = (c * 128 + 118 + np.arange(20) >= 0) & (c * 128 + 118 + np.arange(20) < H)
    for v in range(K):
        for u in range(K):
            p = float(psf[u, v])
            ks = np.arange(u, 128)
            ms = np.arange(0, 128 - u)
            lh[ks, v * 128 + ms] = np.where(livek[ks], p, 0.0)
            ks2 = np.arange(0, 20)
            sel = ks2 + 1 <= u
            ks2 = ks2[sel]
            if ks2.size:
                ms2 = ks2 + 128 - u
                lh[ks2, K * 128 + v * 128 + ms2] = np.where(livek2[ks2], p, 0.0)
    return lh


def _ensure_runtime():
    if 'fn' in _RT:
        return
    import time
    import jax
    import jax.numpy as jnp
    from jax.sharding import Mesh, PartitionSpec, NamedSharding
    from jax.experimental.shard_map import shard_map
    import concourse.mybir as mybir
    from concourse import bass2jax

    t0 = time.perf_counter()
    nc = _build_nc()
    PHASES['build_nc'] = time.perf_counter() - t0

    bass2jax.install_neuronx_cc_hook()

    partition_name = (nc.partition_id_tensor.name
                      if nc.partition_id_tensor is not None else None)
    in_names, out_names, out_avals, zero_shapes = [], [], [], []
    for alloc in nc.m.functions[0].allocations:
        if not isinstance(alloc, mybir.MemoryLocationSet):
            continue
        name = alloc.memorylocations[0].name
        if alloc.kind == "ExternalInput":
            if name != partition_name:
                in_names.append(name)
        elif alloc.kind == "ExternalOutput":
            shape = tuple(alloc.tensor_shape)
            dtype = mybir.dt.np(alloc.dtype)
            out_names.append(name)
            out_avals.append(jax.core.ShapedArray(shape, dtype))
            zero_shapes.append((shape, dtype))
    n_params = len(in_names)
    all_names = in_names + out_names

    devs = jax.devices()[:NCORES]
    mesh = Mesh(np.asarray(devs), ("core",))
    P = PartitionSpec
    nsh = NamedSharding(mesh, P("core"))

    def _body(*args):
        operands = list(args)
        if partition_name is not None:
            operands.append(bass2jax.partition_id_tensor())
        outs = bass2jax._bass_exec_p.bind(
            *operands,
            out_avals=tuple(out_avals),
            in_names=tuple(all_names + ([partition_name] if partition_name else [])),
            out_names=tuple(out_names),
            lowering_input_output_aliases=(),
            sim_require_finite=False,
            sim_require_nnan=False,
            nc=nc,
        )
        return tuple(outs)

    nin = n_params + len(out_names)
    fn = jax.jit(
        shard_map(_body, mesh=mesh,
                  in_specs=(P("core"),) * nin,
                  out_specs=(P("core"),) * len(out_names),
                  check_rep=False),
        donate_argnums=tuple(range(n_params, nin)),
        keep_unused=True,
    )

    zout_fns = []
    for shape, dtype in zero_shapes:
        gshape = (NCORES * shape[0],) + tuple(shape[1:])
        zf = jax.jit(lambda s=gshape, d=dtype: jnp.zeros(s, d), out_shardings=nsh)
        zout_fns.append(zf)

    # persistent dummy src shards for cores 1..7 (content never read)
    dummies = []
    for i in range(1, NCORES):
        df = jax.jit(lambda: jnp.zeros((NPIXC,), jnp.float16),
                     out_shardings=jax.sharding.SingleDeviceSharding(devs[i]))
        dummies.append(df())
    for a in dummies:
        a.block_until_ready()

    _RT.update(nc=nc, fn=fn, mesh=mesh, devs=devs, nsh=nsh,
               in_names=in_names, out_names=out_names,
               zout_fns=zout_fns, dummies=dummies, donors=[], jax=jax, np_mod=np)


def _ensure_geometry(x0, y0, raw_b, raw_rc, raw_subpix, raw_psf):
    import time
    key = (float(x0), float(y0), float(raw_b), float(raw_rc),
           np.asarray(raw_subpix, np.float64).tobytes(),
           np.asarray(raw_psf, np.float64).tobytes())
    if _RT.get('geom_key') == key:
        return
    t0 = time.perf_counter()
    jax = _RT['jax']
    ix0, iy0, wx, wy = _geometry(float(x0), float(y0), float(raw_b),
                                 float(raw_rc), np.asarray(raw_subpix))
    psf = np.maximum(np.asarray(raw_psf, np.float64).reshape(K, K), 0.0)
    psf = psf / max(psf.sum(), 1e-12)
    psf = psf.astype(np.float32)

    soffs, idxs, wtss, lhs = [], [], [], []
    for c in range(NCORES):
        soff, idxp, wts = _pack_core(c, ix0, iy0, wx, wy)
        soffs.append(soff)
        idxs.append(idxp)
        wtss.append(wts)
        lhs.append(_pack_lh(c, psf))
    PHASES['geom_pack'] = time.perf_counter() - t0
    t0 = time.perf_counter()
    nsh = _RT['nsh']
    _RT['soff_g'] = jax.device_put(np.concatenate(soffs, axis=0), nsh)
    _RT['idx_g'] = jax.device_put(np.concatenate(idxs, axis=0), nsh)
    _RT['wts_g'] = jax.device_put(np.concatenate(wtss, axis=0).astype(np.float16), nsh)
    _RT['lh_g'] = jax.device_put(np.concatenate(lhs, axis=0), nsh)
    for a in (_RT['soff_g'], _RT['idx_g'], _RT['wts_g'], _RT['lh_g']):
        a.block_until_ready()
    PHASES['geom_upload'] = time.perf_counter() - t0
    _RT['geom_key'] = key


def _fetch_shard0(arr):
    for sh in arr.addressable_shards:
        if sh.index[0].start in (0, None):
            return np.asarray(sh.data)
    raise RuntimeError("shard0 not found")


def kernel(src, raw_psf, x0, y0, raw_b, raw_rc, raw_subpix):
    global LAST_EXEC_NS
    import time
    t_all0 = time.perf_counter()
    _ensure_runtime()
    _ensure_geometry(x0, y0, raw_b, raw_rc, raw_subpix, raw_psf)
    jax = _RT['jax']
    nsh = _RT['nsh']
    fn = _RT['fn']
    geom = [_RT['soff_g'], _RT['idx_g'], _RT['wts_g'], _RT['lh_g']]
    gmap = dict(zip(['soff', 'idx', 'wts', 'lh'], geom))

    srcv = np.asarray(src, np.float32).reshape(NCALLS, NPIXC)
    PHASES['convert_in'] = 0.0

    # pipelined calls: enqueue everything, then fetch in order (duplex relay
    # overlaps call k+1 upload/exec with call k download)
    t0 = time.perf_counter()
    donors = _RT['donors']
    _RT['donors'] = []
    outs_per_call = []
    oi = _RT['out_names'].index('out')
    for k in range(NCALLS):
        if donors:
            zouts = [donors.pop()]
        else:
            zouts = [zf() for zf in _RT['zout_fns']]
        s0 = jax.device_put(srcv[k].astype(np.float16), _RT['devs'][0])
        src_g = jax.make_array_from_single_device_arrays(
            (NCORES * NPIXC,), nsh, [s0] + _RT['dummies'])
        args = {'src': src_g, **gmap}
        ordered = [args[n] for n in _RT['in_names']] + zouts
        outs_per_call.append(fn(*ordered))
    PHASES['dispatch'] = time.perf_counter() - t0

    t0 = time.perf_counter()
    out = np.empty((B, 1, H, W), np.float32)
    for k, outs in enumerate(outs_per_call):
        flat = _fetch_shard0(outs[oi])
        # single pass: dtype conversion fused into the transposing assignment
        ov = out[k * NIMG:(k + 1) * NIMG, 0].reshape(NIMG, NCORES, 128, W)
        ov[...] = flat.reshape(NCORES, NIMG, 128, W).transpose(1, 0, 2, 3)
    _RT['donors'] = [outs[oi] for outs in outs_per_call]
    PHASES['exec_fetch'] = time.perf_counter() - t0
    PHASES['convert_out'] = 0.0

    del outs_per_call
    import gc
    gc.collect()
    LAST_EXEC_NS = int((time.perf_counter() - t_all0) * 1e9)
    return out


# revision 5
# speedup vs baseline: 1.5894x; 1.3304x over previous
import sys
sys.path.insert(0, '/opt/trn_rl_repo')
import numpy as np

B = 16
H = 1024
W = 1024
K = 21
PAD = 10
NCORES = 8
WR = 148          # warp rows held per core (128 + 2*PAD)
HALF = 74
JCH = 32
NSTEP = 8
NGRP = 8
CPIX = HALF * JCH          # 2368 pixels per chunk
SLAB_R, SLAB_C = 48, 76
SLAB_E = SLAB_R * SLAB_C   # 3648
NI16 = CPIX // 16          # 148 idx cols per gather plane
LHW = 2 * K * 128          # 5376

NIMG = 8                   # images per device call (B / NCALLS)
NCALLS = B // NIMG         # pipelined calls per kernel() invocation
NPIXC = NIMG * H * W       # per-call pixels (8M)
OUTN = NCORES * NIMG * 128 * W
CV = 1024                  # f16->f32 conversion chunk (per partition)
NCV = NPIXC // (128 * CV)  # conversion chunks

LAST_EXEC_NS = None
PHASES = {}

_RT = {}
_TPOOL = None


def _to_f16_threaded(srcv):
    global _TPOOL
    from concurrent.futures import ThreadPoolExecutor
    if _TPOOL is None:
        _TPOOL = ThreadPoolExecutor(8)
    out = np.empty(srcv.size, np.float16)
    n = srcv.size
    step = n // 8
    flat = srcv.reshape(-1)

    def w(i):
        out[i * step:(i + 1) * step] = flat[i * step:(i + 1) * step].astype(np.float16)
    list(_TPOOL.map(w, range(8)))
    return out


def _build_nc():
    import os
    import concourse.bacc as bacc
    import concourse.mybir as mybir
    import concourse.tile as tile
    import concourse.bass as bass
    from contextlib import ExitStack

    f32 = mybir.dt.float32
    f32r = mybir.dt.float32r
    f16 = mybir.dt.float16
    u16 = mybir.dt.uint16
    i32 = mybir.dt.int32
    sub_op = mybir.AluOpType.subtract
    mul_op = mybir.AluOpType.mult
    add_op = mybir.AluOpType.add

    nc = bacc.Bacc(num_devices=NCORES)
    src_d = nc.declare_dram_parameter("src", [NPIXC], f16, isOutput=False)
    soff_d = nc.declare_dram_parameter("soff", [NSTEP, 128, SLAB_R], i32, isOutput=False)
    idx_d = nc.declare_dram_parameter("idx", [NSTEP, 128, 2 * NI16], u16, isOutput=False)
    wts_d = nc.declare_dram_parameter("wts", [NSTEP, 128, 2 * CPIX], f16, isOutput=False)
    lh_d = nc.declare_dram_parameter("lh", [128, LHW], f32r, isOutput=False)
    out_d = nc.declare_dram_parameter("out", [OUTN], f16, isOutput=True)
    RG = [list(range(NCORES))]

    with ExitStack() as ctx:
        tc = ctx.enter_context(tile.TileContext(nc))
        const = ctx.enter_context(tc.tile_pool(name="const", bufs=1))
        dpool = ctx.enter_context(tc.tile_pool(name="dsc", bufs=1, space="DRAM"))
        vpool = ctx.enter_context(tc.tile_pool(name="cvt", bufs=2))
        spool = ctx.enter_context(tc.tile_pool(name="slab", bufs=2))
        ipool = ctx.enter_context(tc.tile_pool(name="idx", bufs=2))
        wpool = ctx.enter_context(tc.tile_pool(name="wts", bufs=2))
        cpool = ctx.enter_context(tc.tile_pool(name="comb", bufs=2))
        gpool = ctx.enter_context(tc.tile_pool(name="gath", bufs=2))
        tpool = ctx.enter_context(tc.tile_pool(name="tmp", bufs=2))
        rpool = ctx.enter_context(tc.tile_pool(name="rhs", bufs=2))
        opool = ctx.enter_context(tc.tile_pool(name="ot", bufs=2))
        pspool = ctx.enter_context(tc.tile_pool(name="ps", bufs=2, space="PSUM"))

        # internal DRAM
        cc_src16i = dpool.tile([NPIXC], f16)
        cc_src16 = dpool.tile([NCORES * NPIXC], f16, addr_space="Shared")
        srcf = dpool.tile([NPIXC, 1], f32)
        cc_outi = dpool.tile([NIMG, 128, W], f16)
        cc_out = dpool.tile([OUTN], f16, addr_space="Shared")
        scratch = dpool.tile([NIMG, WR, W + 2 * PAD], f32r)

        # src block lives on core 0; AllGather copies every core's (mostly
        # garbage) block, and only block 0 -- core 0's real data -- is read
        nc.gpsimd.dma_start(cc_src16i[:], src_d[:])
        nc.gpsimd.collective_compute(
            "AllGather", mybir.AluOpType.bypass, replica_groups=RG,
            ins=[cc_src16i.opt()], outs=[cc_src16.opt()])

        # upconvert f16 -> f32 into srcf
        for k in range(NCV):
            t16 = vpool.tile([128, CV], f16)
            nc.sync.dma_start(t16[:], cc_src16[k * 128 * CV:(k + 1) * 128 * CV])
            t32 = vpool.tile([128, CV], f32)
            nc.vector.tensor_copy(t32[:], t16[:])
            nc.sync.dma_start(srcf[k * 128 * CV:(k + 1) * 128 * CV, :], t32[:])

        lh_t = const.tile([128, LHW], f32r)
        nc.sync.dma_start(lh_t[:], lh_d[:, :])

        zt = const.tile([NIMG, WR, PAD], f32)
        nc.vector.memset(zt[:], 0.0)
        nc.sync.dma_start(scratch[0:NIMG, :, 0:PAD], zt[:].bitcast(f32r))
        nc.sync.dma_start(scratch[0:NIMG, :, W + PAD:W + 2 * PAD], zt[:].bitcast(f32r))

        tt = nc.vector.tensor_tensor

        for s in range(NSTEP):
            soff_t = ipool.tile([128, SLAB_R], i32)
            nc.sync.dma_start(soff_t[:], soff_d[s, :, :])
            slab_t = spool.tile([128, SLAB_E], f32)
            for k in range(SLAB_R):
                nc.gpsimd.indirect_dma_start(
                    out=slab_t[:, k * SLAB_C:(k + 1) * SLAB_C],
                    out_offset=None,
                    in_=srcf[:, :],
                    in_offset=bass.IndirectOffsetOnAxis(ap=soff_t[:, k:k + 1], axis=0))
            slab_v = slab_t[:].rearrange('p (n d) -> p n d', d=2)
            idx_t = ipool.tile([128, 2 * NI16], u16)
            nc.sync.dma_start(idx_t[:], idx_d[s, :, :])
            wts16_t = wpool.tile([128, 2 * CPIX], f16)
            nc.sync.dma_start(wts16_t[:], wts_d[s, :, :])
            wts_t = wpool.tile([128, 2 * CPIX], f32)
            nc.vector.tensor_copy(wts_t[:], wts16_t[:])
            comb_t = cpool.tile([128, CPIX], f32)

            for off, ln in ((0, 1024), (1024, 1024), (2048, 320)):
                G0 = gpool.tile([128, 1024, 2], f32)
                G1 = gpool.tile([128, 1024, 2], f32)
                for q in range(0, ln, 512):
                    sz = min(512, ln - q)
                    o16 = (off + q) // 16
                    nc.gpsimd.indirect_copy(
                        G0[:, q:q + sz, :], slab_v, idx_t[:, o16:o16 + sz // 16],
                        i_know_ap_gather_is_preferred=True)
                    nc.gpsimd.indirect_copy(
                        G1[:, q:q + sz, :], slab_v,
                        idx_t[:, NI16 + o16:NI16 + o16 + sz // 16],
                        i_know_ap_gather_is_preferred=True)
                d_t = tpool.tile([128, 1024], f32)
                x1_t = tpool.tile([128, 1024], f32)
                g00 = G0[:, 0:ln, 0]
                g01 = G0[:, 0:ln, 1]
                g10 = G1[:, 0:ln, 0]
                g11 = G1[:, 0:ln, 1]
                cs = comb_t[:, off:off + ln]
                wxs = wts_t[:, off:off + ln]
                wys = wts_t[:, CPIX + off:CPIX + off + ln]
                dv = d_t[:, 0:ln]
                x1 = x1_t[:, 0:ln]
                tt(dv, g01, g00, op=sub_op)
                tt(dv, dv, wxs, op=mul_op)
                tt(cs, g00, dv, op=add_op)
                tt(dv, g11, g10, op=sub_op)
                tt(dv, dv, wxs, op=mul_op)
                tt(x1, g10, dv, op=add_op)
                tt(x1, x1, cs, op=sub_op)
                tt(x1, x1, wys, op=mul_op)
                tt(cs, cs, x1, op=add_op)

            for g in range(NGRP):
                h, jc = g // 4, 4 * s + (g % 4)
                nc.sync.dma_start(
                    scratch[0:NIMG, HALF * h:HALF * h + HALF,
                            PAD + JCH * jc:PAD + JCH * jc + JCH],
                    comb_t[16 * g:16 * g + NIMG, :].bitcast(f32r))

        for img in range(NIMG):
            for jh in range(2):
                rhs = rpool.tile([128, 2 * 532], f32r)
                nc.sync.dma_start(rhs[0:128, 0:532],
                                  scratch[img, 0:128, 512 * jh:512 * jh + 532])
                nc.sync.dma_start(rhs[0:20, 532:1064],
                                  scratch[img, 128:148, 512 * jh:512 * jh + 532])
                ps = pspool.tile([128, 512], mybir.dt.float32)
                for v in range(K):
                    nc.tensor.matmul(ps[:], lh_t[0:128, 128 * v:128 * v + 128],
                                     rhs[0:128, v:v + 512],
                                     start=(v == 0), stop=False)
                    nc.tensor.matmul(ps[:],
                                     lh_t[0:20, K * 128 + 128 * v:K * 128 + 128 * v + 128],
                                     rhs[0:20, 532 + v:532 + v + 512],
                                     start=False, stop=(v == K - 1))
                ot = opool.tile([128, 512], f16)
                nc.scalar.copy(ot[:], ps[:])
                nc.sync.dma_start(cc_outi[img, :, 512 * jh:512 * jh + 512], ot[:])

        # gather all cores' bands everywhere, write out
        nc.gpsimd.collective_compute(
            "AllGather", mybir.AluOpType.bypass, replica_groups=RG,
            ins=[cc_outi.opt()], outs=[cc_out.opt()])
        nc.gpsimd.dma_start(out_d[:], cc_out[:])

    nc.finalize()
    return nc


def _geometry(x0, y0, raw_b, raw_rc, raw_subpix):
    b = np.log1p(np.exp(np.float64(raw_b))) + 1e-8
    rc = np.log1p(np.exp(np.float64(raw_rc))) + 1e-8
    sub = 0.25 * np.tanh(np.asarray(raw_subpix, np.float64))
    xs = np.linspace(-1.0, 1.0, W)
    ys = np.linspace(-1.0, 1.0, H)
    dx = xs - np.float64(x0)
    dy = ys - np.float64(y0)
    denom = np.sqrt(dx[:, None] ** 2 + dy[None, :] ** 2 + 1e-12 + rc * rc)
    gx = xs[:, None] - b * dx[:, None] / denom + sub[0]
    gy = ys[None, :] - b * dy[None, :] / denom + sub[1]
    ix = (gx + 1.0) * 0.5 * (W - 1)
    iy = (gy + 1.0) * 0.5 * (H - 1)
    ix0 = np.floor(ix).astype(np.int64)
    iy0 = np.floor(iy).astype(np.int64)
    wx = (ix - ix0).astype(np.float32)
    wy = (iy - iy0).astype(np.float32)
    assert ix0.min() >= 0 and ix0.max() + 1 <= W - 1
    assert iy0.min() >= 0 and iy0.max() + 1 <= H - 1
    return ix0, iy0, wx, wy


def _pack_core(c, ix0, iy0, wx, wy):
    rows = np.clip(np.arange(c * 128 - PAD, c * 128 - PAD + WR), 0, H - 1)
    IX0 = ix0[rows, :]
    IY0 = iy0[rows, :]
    WX = wx[rows, :]
    WY = wy[rows, :]
    soff = np.zeros((NSTEP, 128, SLAB_R), np.int32)
    idxp = np.empty((NSTEP, 128, 2 * NI16), np.uint16)
    wts = np.empty((NSTEP, 128, 2 * CPIX), np.float32)
    karr = np.arange(SLAB_R)
    for s in range(NSTEP):
        for g in range(NGRP):
            h, jc = g // 4, 4 * s + (g % 4)
            ksl = slice(HALF * h, HALF * h + HALF)
            jsl = slice(JCH * jc, JCH * jc + JCH)
            cy0 = IY0[ksl, jsl]
            cx0 = IX0[ksl, jsl]
            r0 = int(cy0.min())
            c0 = int(cx0.min())
            assert int(cy0.max()) + 1 - r0 <= SLAB_R - 1, "slab rows overflow"
            assert int(cx0.max()) + 1 - c0 <= SLAB_C - 1, "slab cols overflow"
            r0 = min(r0, H - SLAB_R)
            c0 = min(c0, W - SLAB_C)
            for img in range(NIMG):
                soff[s, 16 * g + img, :] = img * H * W + (r0 + karr) * W + c0
            fl0 = ((cy0 - r0) * SLAB_C + (cx0 - c0)).reshape(CPIX)
            idxp[s, 16 * g:16 * g + 16, 0:NI16] = \
                fl0.reshape(NI16, 16).T.astype(np.uint16)
            idxp[s, 16 * g:16 * g + 16, NI16:] = \
                (fl0 + SLAB_C).reshape(NI16, 16).T.astype(np.uint16)
            wts[s, 16 * g:16 * g + 16, 0:CPIX] = WX[ksl, jsl].reshape(1, CPIX)
            wts[s, 16 * g:16 * g + 16, CPIX:] = WY[ksl, jsl].reshape(1, CPIX)
    return soff, idxp, wts


def _pack_lh(c, psf):
    lh = np.zeros((128, LHW), np.float32)
    livek = (c * 128 - PAD + np.arange(128) >= 0) & (c * 128 - PAD + np.arange(128) < H)
    livek2 = (c * 128 + 118 + np.arange(20) >= 0) & (c * 128 + 118 + np.arange(20) < H)
    for v in range(K):
        for u in range(K):
            p = float(psf[u, v])
            ks = np.arange(u, 128)
            ms = np.arange(0, 128 - u)
            lh[ks, v * 128 + ms] = np.where(livek[ks], p, 0.0)
            ks2 = np.arange(0, 20)
            sel = ks2 + 1 <= u
            ks2 = ks2[sel]
            if ks2.size:
                ms2 = ks2 + 128 - u
                lh[ks2, K * 128 + v * 128 + ms2] = np.where(livek2[ks2], p, 0.0)
    return lh


def _ensure_runtime():
    if 'fn' in _RT:
        return
    import time
    import jax
    import jax.numpy as jnp
    from jax.sharding import Mesh, PartitionSpec, NamedSharding
    from jax.experimental.shard_map import shard_map
    import concourse.mybir as mybir
    from concourse import bass2jax

    t0 = time.perf_counter()
    nc = _build_nc()
    PHASES['build_nc'] = time.perf_counter() - t0

    bass2jax.install_neuronx_cc_hook()

    partition_name = (nc.partition_id_tensor.name
                      if nc.partition_id_tensor is not None else None)
    in_names, out_names, out_avals, zero_shapes = [], [], [], []
    for alloc in nc.m.functions[0].allocations:
        if not isinstance(alloc, mybir.MemoryLocationSet):
            continue
        name = alloc.memorylocations[0].name
        if alloc.kind == "ExternalInput":
            if name != partition_name:
                in_names.append(name)
        elif alloc.kind == "ExternalOutput":
            shape = tuple(alloc.tensor_shape)
            dtype = mybir.dt.np(alloc.dtype)
            out_names.append(name)
            out_avals.append(jax.core.ShapedArray(shape, dtype))
            zero_shapes.append((shape, dtype))
    n_params = len(in_names)
    all_names = in_names + out_names

    devs = jax.devices()[:NCORES]
    mesh = Mesh(np.asarray(devs), ("core",))
    P = PartitionSpec
    nsh = NamedSharding(mesh, P("core"))

    def _body(*args):
        operands = list(args)
        if partition_name is not None:
            operands.append(bass2jax.partition_id_tensor())
        outs = bass2jax._bass_exec_p.bind(
            *operands,
            out_avals=tuple(out_avals),
            in_names=tuple(all_names + ([partition_name] if partition_name else [])),
            out_names=tuple(out_names),
            lowering_input_output_aliases=(),
            sim_require_finite=False,
            sim_require_nnan=False,
            nc=nc,
        )
        return tuple(outs)

    nin = n_params + len(out_names)
    fn = jax.jit(
        shard_map(_body, mesh=mesh,
                  in_specs=(P("core"),) * nin,
                  out_specs=(P("core"),) * len(out_names),
                  check_rep=False),
        donate_argnums=tuple(range(n_params, nin)),
        keep_unused=True,
    )

    zout_fns = []
    for shape, dtype in zero_shapes:
        gshape = (NCORES * shape[0],) + tuple(shape[1:])
        zf = jax.jit(lambda s=gshape, d=dtype: jnp.zeros(s, d), out_shardings=nsh)
        zout_fns.append(zf)

    # persistent dummy src shards for cores 1..7 (content never read)
    dummies = []
    for i in range(1, NCORES):
        df = jax.jit(lambda: jnp.zeros((NPIXC,), jnp.float16),
                     out_shardings=jax.sharding.SingleDeviceSharding(devs[i]))
        dummies.append(df())
    for a in dummies:
        a.block_until_ready()

    _RT.update(nc=nc, fn=fn, mesh=mesh, devs=devs, nsh=nsh,
               in_names=in_names, out_names=out_names,
               zout_fns=zout_fns, dummies=dummies, donors=[], jax=jax, np_mod=np)


def _ensure_geometry(x0, y0, raw_b, raw_rc, raw_subpix, raw_psf):
    import time
    key = (float(x0), float(y0), float(raw_b), float(raw_rc),
           np.asarray(raw_subpix, np.float64).tobytes(),
           np.asarray(raw_psf, np.float64).tobytes())
    if _RT.get('geom_key') == key:
        return
    t0 = time.perf_counter()
    jax = _RT['jax']
    ix0, iy0, wx, wy = _geometry(float(x0), float(y0), float(raw_b),
                                 float(raw_rc), np.asarray(raw_subpix))
    psf = np.maximum(np.asarray(raw_psf, np.float64).reshape(K, K), 0.0)
    psf = psf / max(psf.sum(), 1e-12)
    psf = psf.astype(np.float32)

    soffs, idxs, wtss, lhs = [], [], [], []
    for c in range(NCORES):
        soff, idxp, wts = _pack_core(c, ix0, iy0, wx, wy)
        soffs.append(soff)
        idxs.append(idxp)
        wtss.append(wts)
        lhs.append(_pack_lh(c, psf))
    PHASES['geom_pack'] = time.perf_counter() - t0
    t0 = time.perf_counter()
    nsh = _RT['nsh']
    _RT['soff_g'] = jax.device_put(np.concatenate(soffs, axis=0), nsh)
    _RT['idx_g'] = jax.device_put(np.concatenate(idxs, axis=0), nsh)
    _RT['wts_g'] = jax.device_put(np.concatenate(wtss, axis=0).astype(np.float16), nsh)
    _RT['lh_g'] = jax.device_put(np.concatenate(lhs, axis=0), nsh)
    for a in (_RT['soff_g'], _RT['idx_g'], _RT['wts_g'], _RT['lh_g']):
        a.block_until_ready()
    PHASES['geom_upload'] = time.perf_counter() - t0
    _RT['geom_key'] = key


def _fetch_shard0(arr):
    for sh in arr.addressable_shards:
        if sh.index[0].start in (0, None):
            return np.asarray(sh.data)
    raise RuntimeError("shard0 not found")


def kernel(src, raw_psf, x0, y0, raw_b, raw_rc, raw_subpix):
    global LAST_EXEC_NS
    import time
    t_all0 = time.perf_counter()
    _ensure_runtime()
    _ensure_geometry(x0, y0, raw_b, raw_rc, raw_subpix, raw_psf)
    jax = _RT['jax']
    nsh = _RT['nsh']
    fn = _RT['fn']
    geom = [_RT['soff_g'], _RT['idx_g'], _RT['wts_g'], _RT['lh_g']]
    gmap = dict(zip(['soff', 'idx', 'wts', 'lh'], geom))

    srcv = np.asarray(src, np.float32).reshape(NCALLS, NPIXC)
    PHASES['convert_in'] = 0.0

    # pipelined calls: enqueue everything, then fetch in order (duplex relay
    # overlaps call k+1 upload/exec with call k download)
    t0 = time.perf_counter()
    donors = _RT['donors']
    _RT['donors'] = []
    outs_per_call = []
    oi = _RT['out_names'].index('out')
    for k in range(NCALLS):
        if donors:
            zouts = [donors.pop()]
        else:
            zouts = [zf() for zf in _RT['zout_fns']]
        s0 = jax.device_put(srcv[k].astype(np.float16), _RT['devs'][0])
        src_g = jax.make_array_from_single_device_arrays(
            (NCORES * NPIXC,), nsh, [s0] + _RT['dummies'])
        args = {'src': src_g, **gmap}
        ordered = [args[n] for n in _RT['in_names']] + zouts
        outs_per_call.append(fn(*ordered))
    PHASES['dispatch'] = time.perf_counter() - t0

    t0 = time.perf_counter()
    out = np.empty((B, 1, H, W), np.float32)
    for k, outs in enumerate(outs_per_call):
        flat = _fetch_shard0(outs[oi])
        # single pass: dtype conversion fused into the transposing assignment
        ov = out[k * NIMG:(k + 1) * NIMG, 0].reshape(NIMG, NCORES, 128, W)
        ov[...] = flat.reshape(NCORES, NIMG, 128, W).transpose(1, 0, 2, 3)
    _RT['donors'] = [outs[oi] for outs in outs_per_call]
    PHASES['exec_fetch'] = time.perf_counter() - t0
    PHASES['convert_out'] = 0.0

    del outs_per_call
    _RT['ncalls_done'] = _RT.get('ncalls_done', 0) + 1
    if _RT['ncalls_done'] % 4 == 0:
        import gc
        gc.collect()
    LAST_EXEC_NS = int((time.perf_counter() - t_all0) * 1e9)
    return out


# revision 6
# speedup vs baseline: 1.6892x; 1.0628x over previous
import sys
sys.path.insert(0, '/opt/trn_rl_repo')
import numpy as np

B = 16
H = 1024
W = 1024
K = 21
PAD = 10
NCORES = 8
WR = 148          # warp rows held per core (128 + 2*PAD)
HALF = 74
JCH = 32
NSTEP = 8
NGRP = 8
CPIX = HALF * JCH          # 2368 pixels per chunk
SLAB_R, SLAB_C = 48, 76
SLAB_E = SLAB_R * SLAB_C   # 3648
NI16 = CPIX // 16          # 148 idx cols per gather plane
LHW = 2 * K * 128          # 5376

NIMG = 8                   # images per device call (B / NCALLS)
NCALLS = B // NIMG         # pipelined calls per kernel() invocation
NPIXC = NIMG * H * W       # per-call pixels (8M)
OUTN = NCORES * NIMG * 128 * W
CV = 1024                  # f16->f32 conversion chunk (per partition)
NCV = NPIXC // (128 * CV)  # conversion chunks

LAST_EXEC_NS = None
PHASES = {}

_RT = {}
_TPOOL = None


def _to_f16_threaded(srcv):
    global _TPOOL
    from concurrent.futures import ThreadPoolExecutor
    if _TPOOL is None:
        _TPOOL = ThreadPoolExecutor(8)
    out = np.empty(srcv.size, np.float16)
    n = srcv.size
    step = n // 8
    flat = srcv.reshape(-1)

    def w(i):
        out[i * step:(i + 1) * step] = flat[i * step:(i + 1) * step].astype(np.float16)
    list(_TPOOL.map(w, range(8)))
    return out


def _build_nc():
    import os
    import concourse.bacc as bacc
    import concourse.mybir as mybir
    import concourse.tile as tile
    import concourse.bass as bass
    from contextlib import ExitStack

    f32 = mybir.dt.float32
    f32r = mybir.dt.float32r
    f16 = mybir.dt.float16
    u16 = mybir.dt.uint16
    i32 = mybir.dt.int32
    sub_op = mybir.AluOpType.subtract
    mul_op = mybir.AluOpType.mult
    add_op = mybir.AluOpType.add

    nc = bacc.Bacc(num_devices=NCORES)
    src_d = nc.declare_dram_parameter("src", [NPIXC], f16, isOutput=False)
    soff_d = nc.declare_dram_parameter("soff", [NSTEP, 128, SLAB_R], i32, isOutput=False)
    idx_d = nc.declare_dram_parameter("idx", [NSTEP, 128, 2 * NI16], u16, isOutput=False)
    wts_d = nc.declare_dram_parameter("wts", [NSTEP, 128, 2 * CPIX], f16, isOutput=False)
    lh_d = nc.declare_dram_parameter("lh", [128, LHW], f32r, isOutput=False)
    out_d = nc.declare_dram_parameter("out", [OUTN], f16, isOutput=True)
    RG = [list(range(NCORES))]

    with ExitStack() as ctx:
        tc = ctx.enter_context(tile.TileContext(nc))
        const = ctx.enter_context(tc.tile_pool(name="const", bufs=1))
        dpool = ctx.enter_context(tc.tile_pool(name="dsc", bufs=1, space="DRAM"))
        vpool = ctx.enter_context(tc.tile_pool(name="cvt", bufs=2))
        spool = ctx.enter_context(tc.tile_pool(name="slab", bufs=2))
        ipool = ctx.enter_context(tc.tile_pool(name="idx", bufs=2))
        wpool = ctx.enter_context(tc.tile_pool(name="wts", bufs=2))
        cpool = ctx.enter_context(tc.tile_pool(name="comb", bufs=2))
        gpool = ctx.enter_context(tc.tile_pool(name="gath", bufs=2))
        tpool = ctx.enter_context(tc.tile_pool(name="tmp", bufs=2))
        rpool = ctx.enter_context(tc.tile_pool(name="rhs", bufs=2))
        opool = ctx.enter_context(tc.tile_pool(name="ot", bufs=2))
        pspool = ctx.enter_context(tc.tile_pool(name="ps", bufs=2, space="PSUM"))

        # internal DRAM
        cc_src16i = dpool.tile([NPIXC], f16)
        cc_src16 = dpool.tile([NCORES * NPIXC], f16, addr_space="Shared")
        srcf = dpool.tile([NPIXC, 1], f32)
        cc_outi = dpool.tile([NIMG, 128, W], f16)
        cc_out = dpool.tile([OUTN], f16, addr_space="Shared")
        scratch = dpool.tile([NIMG, WR, W + 2 * PAD], f32r)

        # src block lives on core 0; AllGather copies every core's (mostly
        # garbage) block, and only block 0 -- core 0's real data -- is read
        nc.gpsimd.dma_start(cc_src16i[:], src_d[:])
        nc.gpsimd.collective_compute(
            "AllGather", mybir.AluOpType.bypass, replica_groups=RG,
            ins=[cc_src16i.opt()], outs=[cc_src16.opt()])

        # upconvert f16 -> f32 into srcf
        for k in range(NCV):
            t16 = vpool.tile([128, CV], f16)
            nc.sync.dma_start(t16[:], cc_src16[k * 128 * CV:(k + 1) * 128 * CV])
            t32 = vpool.tile([128, CV], f32)
            nc.vector.tensor_copy(t32[:], t16[:])
            nc.sync.dma_start(srcf[k * 128 * CV:(k + 1) * 128 * CV, :], t32[:])

        lh_t = const.tile([128, LHW], f32r)
        nc.sync.dma_start(lh_t[:], lh_d[:, :])

        zt = const.tile([NIMG, WR, PAD], f32)
        nc.vector.memset(zt[:], 0.0)
        nc.sync.dma_start(scratch[0:NIMG, :, 0:PAD], zt[:].bitcast(f32r))
        nc.sync.dma_start(scratch[0:NIMG, :, W + PAD:W + 2 * PAD], zt[:].bitcast(f32r))

        tt = nc.vector.tensor_tensor

        for s in range(NSTEP):
            soff_t = ipool.tile([128, SLAB_R], i32)
            nc.sync.dma_start(soff_t[:], soff_d[s, :, :])
            slab_t = spool.tile([128, SLAB_E], f32)
            for k in range(SLAB_R):
                nc.gpsimd.indirect_dma_start(
                    out=slab_t[:, k * SLAB_C:(k + 1) * SLAB_C],
                    out_offset=None,
                    in_=srcf[:, :],
                    in_offset=bass.IndirectOffsetOnAxis(ap=soff_t[:, k:k + 1], axis=0))
            slab_v = slab_t[:].rearrange('p (n d) -> p n d', d=2)
            idx_t = ipool.tile([128, 2 * NI16], u16)
            nc.sync.dma_start(idx_t[:], idx_d[s, :, :])
            wts16_t = wpool.tile([128, 2 * CPIX], f16)
            nc.sync.dma_start(wts16_t[:], wts_d[s, :, :])
            wts_t = wpool.tile([128, 2 * CPIX], f32)
            nc.vector.tensor_copy(wts_t[:], wts16_t[:])
            comb_t = cpool.tile([128, CPIX], f32)

            for off, ln in ((0, 1024), (1024, 1024), (2048, 320)):
                G0 = gpool.tile([128, 1024, 2], f32)
                G1 = gpool.tile([128, 1024, 2], f32)
                for q in range(0, ln, 512):
                    sz = min(512, ln - q)
                    o16 = (off + q) // 16
                    nc.gpsimd.indirect_copy(
                        G0[:, q:q + sz, :], slab_v, idx_t[:, o16:o16 + sz // 16],
                        i_know_ap_gather_is_preferred=True)
                    nc.gpsimd.indirect_copy(
                        G1[:, q:q + sz, :], slab_v,
                        idx_t[:, NI16 + o16:NI16 + o16 + sz // 16],
                        i_know_ap_gather_is_preferred=True)
                d_t = tpool.tile([128, 1024], f32)
                x1_t = tpool.tile([128, 1024], f32)
                g00 = G0[:, 0:ln, 0]
                g01 = G0[:, 0:ln, 1]
                g10 = G1[:, 0:ln, 0]
                g11 = G1[:, 0:ln, 1]
                cs = comb_t[:, off:off + ln]
                wxs = wts_t[:, off:off + ln]
                wys = wts_t[:, CPIX + off:CPIX + off + ln]
                dv = d_t[:, 0:ln]
                x1 = x1_t[:, 0:ln]
                tt(dv, g01, g00, op=sub_op)
                tt(dv, dv, wxs, op=mul_op)
                tt(cs, g00, dv, op=add_op)
                tt(dv, g11, g10, op=sub_op)
                tt(dv, dv, wxs, op=mul_op)
                tt(x1, g10, dv, op=add_op)
                tt(x1, x1, cs, op=sub_op)
                tt(x1, x1, wys, op=mul_op)
                tt(cs, cs, x1, op=add_op)

            for g in range(NGRP):
                h, jc = g // 4, 4 * s + (g % 4)
                nc.sync.dma_start(
                    scratch[0:NIMG, HALF * h:HALF * h + HALF,
                            PAD + JCH * jc:PAD + JCH * jc + JCH],
                    comb_t[16 * g:16 * g + NIMG, :].bitcast(f32r))

        for img in range(NIMG):
            for jh in range(2):
                rhs = rpool.tile([128, 2 * 532], f32r)
                nc.sync.dma_start(rhs[0:128, 0:532],
                                  scratch[img, 0:128, 512 * jh:512 * jh + 532])
                nc.sync.dma_start(rhs[0:20, 532:1064],
                                  scratch[img, 128:148, 512 * jh:512 * jh + 532])
                ps = pspool.tile([128, 512], mybir.dt.float32)
                for v in range(K):
                    nc.tensor.matmul(ps[:], lh_t[0:128, 128 * v:128 * v + 128],
                                     rhs[0:128, v:v + 512],
                                     start=(v == 0), stop=False)
                    nc.tensor.matmul(ps[:],
                                     lh_t[0:20, K * 128 + 128 * v:K * 128 + 128 * v + 128],
                                     rhs[0:20, 532 + v:532 + v + 512],
                                     start=False, stop=(v == K - 1))
                ot = opool.tile([128, 512], f16)
                nc.scalar.copy(ot[:], ps[:])
                nc.sync.dma_start(cc_outi[img, :, 512 * jh:512 * jh + 512], ot[:])

        # gather all cores' bands everywhere, write out
        nc.gpsimd.collective_compute(
            "AllGather", mybir.AluOpType.bypass, replica_groups=RG,
            ins=[cc_outi.opt()], outs=[cc_out.opt()])
        nc.gpsimd.dma_start(out_d[:], cc_out[:])

    nc.finalize()
    return nc


def _geometry(x0, y0, raw_b, raw_rc, raw_subpix):
    b = np.log1p(np.exp(np.float64(raw_b))) + 1e-8
    rc = np.log1p(np.exp(np.float64(raw_rc))) + 1e-8
    sub = 0.25 * np.tanh(np.asarray(raw_subpix, np.float64))
    xs = np.linspace(-1.0, 1.0, W)
    ys = np.linspace(-1.0, 1.0, H)
    dx = xs - np.float64(x0)
    dy = ys - np.float64(y0)
    denom = np.sqrt(dx[:, None] ** 2 + dy[None, :] ** 2 + 1e-12 + rc * rc)
    gx = xs[:, None] - b * dx[:, None] / denom + sub[0]
    gy = ys[None, :] - b * dy[None, :] / denom + sub[1]
    ix = (gx + 1.0) * 0.5 * (W - 1)
    iy = (gy + 1.0) * 0.5 * (H - 1)
    ix0 = np.floor(ix).astype(np.int64)
    iy0 = np.floor(iy).astype(np.int64)
    wx = (ix - ix0).astype(np.float32)
    wy = (iy - iy0).astype(np.float32)
    assert ix0.min() >= 0 and ix0.max() + 1 <= W - 1
    assert iy0.min() >= 0 and iy0.max() + 1 <= H - 1
    return ix0, iy0, wx, wy


def _pack_core(c, ix0, iy0, wx, wy):
    rows = np.clip(np.arange(c * 128 - PAD, c * 128 - PAD + WR), 0, H - 1)
    IX0 = ix0[rows, :]
    IY0 = iy0[rows, :]
    WX = wx[rows, :]
    WY = wy[rows, :]
    soff = np.zeros((NSTEP, 128, SLAB_R), np.int32)
    idxp = np.empty((NSTEP, 128, 2 * NI16), np.uint16)
    wts = np.empty((NSTEP, 128, 2 * CPIX), np.float32)
    karr = np.arange(SLAB_R)
    for s in range(NSTEP):
        for g in range(NGRP):
            h, jc = g // 4, 4 * s + (g % 4)
            ksl = slice(HALF * h, HALF * h + HALF)
            jsl = slice(JCH * jc, JCH * jc + JCH)
            cy0 = IY0[ksl, jsl]
            cx0 = IX0[ksl, jsl]
            r0 = int(cy0.min())
            c0 = int(cx0.min())
            assert int(cy0.max()) + 1 - r0 <= SLAB_R - 1, "slab rows overflow"
            assert int(cx0.max()) + 1 - c0 <= SLAB_C - 1, "slab cols overflow"
            r0 = min(r0, H - SLAB_R)
            c0 = min(c0, W - SLAB_C)
            for img in range(NIMG):
                soff[s, 16 * g + img, :] = img * H * W + (r0 + karr) * W + c0
            fl0 = ((cy0 - r0) * SLAB_C + (cx0 - c0)).reshape(CPIX)
            idxp[s, 16 * g:16 * g + 16, 0:NI16] = \
                fl0.reshape(NI16, 16).T.astype(np.uint16)
            idxp[s, 16 * g:16 * g + 16, NI16:] = \
                (fl0 + SLAB_C).reshape(NI16, 16).T.astype(np.uint16)
            wts[s, 16 * g:16 * g + 16, 0:CPIX] = WX[ksl, jsl].reshape(1, CPIX)
            wts[s, 16 * g:16 * g + 16, CPIX:] = WY[ksl, jsl].reshape(1, CPIX)
    return soff, idxp, wts


def _pack_lh(c, psf):
    lh = np.zeros((128, LHW), np.float32)
    livek = (c * 128 - PAD + np.arange(128) >= 0) & (c * 128 - PAD + np.arange(128) < H)
    livek2 = (c * 128 + 118 + np.arange(20) >= 0) & (c * 128 + 118 + np.arange(20) < H)
    for v in range(K):
        for u in range(K):
            p = float(psf[u, v])
            ks = np.arange(u, 128)
            ms = np.arange(0, 128 - u)
            lh[ks, v * 128 + ms] = np.where(livek[ks], p, 0.0)
            ks2 = np.arange(0, 20)
            sel = ks2 + 1 <= u
            ks2 = ks2[sel]
            if ks2.size:
                ms2 = ks2 + 128 - u
                lh[ks2, K * 128 + v * 128 + ms2] = np.where(livek2[ks2], p, 0.0)
    return lh


def _ensure_runtime():
    if 'fn' in _RT:
        return
    import time
    import jax
    import jax.numpy as jnp
    from jax.sharding import Mesh, PartitionSpec, NamedSharding
    from jax.experimental.shard_map import shard_map
    import concourse.mybir as mybir
    from concourse import bass2jax

    t0 = time.perf_counter()
    nc = _build_nc()
    PHASES['build_nc'] = time.perf_counter() - t0

    bass2jax.install_neuronx_cc_hook()

    partition_name = (nc.partition_id_tensor.name
                      if nc.partition_id_tensor is not None else None)
    in_names, out_names, out_avals, zero_shapes = [], [], [], []
    for alloc in nc.m.functions[0].allocations:
        if not isinstance(alloc, mybir.MemoryLocationSet):
            continue
        name = alloc.memorylocations[0].name
        if alloc.kind == "ExternalInput":
            if name != partition_name:
                in_names.append(name)
        elif alloc.kind == "ExternalOutput":
            shape = tuple(alloc.tensor_shape)
            dtype = mybir.dt.np(alloc.dtype)
            out_names.append(name)
            out_avals.append(jax.core.ShapedArray(shape, dtype))
            zero_shapes.append((shape, dtype))
    n_params = len(in_names)
    all_names = in_names + out_names

    devs = jax.devices()[:NCORES]
    mesh = Mesh(np.asarray(devs), ("core",))
    P = PartitionSpec
    nsh = NamedSharding(mesh, P("core"))

    def _body(*args):
        operands = list(args)
        if partition_name is not None:
            operands.append(bass2jax.partition_id_tensor())
        outs = bass2jax._bass_exec_p.bind(
            *operands,
            out_avals=tuple(out_avals),
            in_names=tuple(all_names + ([partition_name] if partition_name else [])),
            out_names=tuple(out_names),
            lowering_input_output_aliases=(),
            sim_require_finite=False,
            sim_require_nnan=False,
            nc=nc,
        )
        return tuple(outs)

    nin = n_params + len(out_names)
    fn = jax.jit(
        shard_map(_body, mesh=mesh,
                  in_specs=(P("core"),) * nin,
                  out_specs=(P("core"),) * len(out_names),
                  check_rep=False),
        donate_argnums=tuple(range(n_params, nin)),
        keep_unused=True,
    )

    zout_fns = []
    for shape, dtype in zero_shapes:
        gshape = (NCORES * shape[0],) + tuple(shape[1:])
        zf = jax.jit(lambda s=gshape, d=dtype: jnp.zeros(s, d), out_shardings=nsh)
        zout_fns.append(zf)

    # persistent dummy src shards for cores 1..7 (content never read)
    dummies = []
    for i in range(1, NCORES):
        df = jax.jit(lambda: jnp.zeros((NPIXC,), jnp.float16),
                     out_shardings=jax.sharding.SingleDeviceSharding(devs[i]))
        dummies.append(df())
    for a in dummies:
        a.block_until_ready()

    _RT.update(nc=nc, fn=fn, mesh=mesh, devs=devs, nsh=nsh,
               in_names=in_names, out_names=out_names,
               zout_fns=zout_fns, dummies=dummies, donors=[], jax=jax, np_mod=np)


def _ensure_geometry(x0, y0, raw_b, raw_rc, raw_subpix, raw_psf):
    import time
    key = (float(x0), float(y0), float(raw_b), float(raw_rc),
           np.asarray(raw_subpix, np.float64).tobytes(),
           np.asarray(raw_psf, np.float64).tobytes())
    if _RT.get('geom_key') == key:
        return
    t0 = time.perf_counter()
    jax = _RT['jax']
    ix0, iy0, wx, wy = _geometry(float(x0), float(y0), float(raw_b),
                                 float(raw_rc), np.asarray(raw_subpix))
    psf = np.maximum(np.asarray(raw_psf, np.float64).reshape(K, K), 0.0)
    psf = psf / max(psf.sum(), 1e-12)
    psf = psf.astype(np.float32)

    soffs, idxs, wtss, lhs = [], [], [], []
    for c in range(NCORES):
        soff, idxp, wts = _pack_core(c, ix0, iy0, wx, wy)
        soffs.append(soff)
        idxs.append(idxp)
        wtss.append(wts)
        lhs.append(_pack_lh(c, psf))
    PHASES['geom_pack'] = time.perf_counter() - t0
    t0 = time.perf_counter()
    nsh = _RT['nsh']
    _RT['soff_g'] = jax.device_put(np.concatenate(soffs, axis=0), nsh)
    _RT['idx_g'] = jax.device_put(np.concatenate(idxs, axis=0), nsh)
    _RT['wts_g'] = jax.device_put(np.concatenate(wtss, axis=0).astype(np.float16), nsh)
    _RT['lh_g'] = jax.device_put(np.concatenate(lhs, axis=0), nsh)
    for a in (_RT['soff_g'], _RT['idx_g'], _RT['wts_g'], _RT['lh_g']):
        a.block_until_ready()
    PHASES['geom_upload'] = time.perf_counter() - t0
    _RT['geom_key'] = key


def _fetch_shard0(arr):
    for sh in arr.addressable_shards:
        if sh.index[0].start in (0, None):
            return np.asarray(sh.data)
    raise RuntimeError("shard0 not found")


def kernel(src, raw_psf, x0, y0, raw_b, raw_rc, raw_subpix):
    global LAST_EXEC_NS
    import time
    t_all0 = time.perf_counter()
    _ensure_runtime()
    _ensure_geometry(x0, y0, raw_b, raw_rc, raw_subpix, raw_psf)
    jax = _RT['jax']
    nsh = _RT['nsh']
    fn = _RT['fn']
    geom = [_RT['soff_g'], _RT['idx_g'], _RT['wts_g'], _RT['lh_g']]
    gmap = dict(zip(['soff', 'idx', 'wts', 'lh'], geom))

    srcv = np.asarray(src, np.float32).reshape(NCALLS, NPIXC)
    PHASES['convert_in'] = 0.0

    # pipelined calls: enqueue everything, then fetch in order (duplex relay
    # overlaps call k+1 upload/exec with call k download)
    t0 = time.perf_counter()
    donors = _RT['donors']
    _RT['donors'] = []
    outs_per_call = []
    oi = _RT['out_names'].index('out')
    for k in range(NCALLS):
        if donors:
            zouts = [donors.pop()]
        else:
            zouts = [zf() for zf in _RT['zout_fns']]
        s0 = jax.device_put(srcv[k].astype(np.float16), _RT['devs'][0])
        src_g = jax.make_array_from_single_device_arrays(
            (NCORES * NPIXC,), nsh, [s0] + _RT['dummies'])
        args = {'src': src_g, **gmap}
        ordered = [args[n] for n in _RT['in_names']] + zouts
        outs_per_call.append(fn(*ordered))
    PHASES['dispatch'] = time.perf_counter() - t0

    t0 = time.perf_counter()
    from concurrent.futures import ThreadPoolExecutor
    if _RT.get('fpool') is None:
        _RT['fpool'] = ThreadPoolExecutor(NCALLS)
    # enqueue both fetches immediately so call B's download queues on the
    # relay right behind call A's instead of after A's host-side convert
    futs = [_RT['fpool'].submit(_fetch_shard0, outs[oi]) for outs in outs_per_call]
    out = np.empty((B, 1, H, W), np.float32)
    for k, fut in enumerate(futs):
        flat = fut.result()
        # single pass: dtype conversion fused into the transposing assignment
        ov = out[k * NIMG:(k + 1) * NIMG, 0].reshape(NIMG, NCORES, 128, W)
        ov[...] = flat.reshape(NCORES, NIMG, 128, W).transpose(1, 0, 2, 3)
    _RT['donors'] = [outs[oi] for outs in outs_per_call]
    PHASES['exec_fetch'] = time.perf_counter() - t0
    PHASES['convert_out'] = 0.0

    del outs_per_call
    _RT['ncalls_done'] = _RT.get('ncalls_done', 0) + 1
    if _RT['ncalls_done'] % 4 == 0:
        import gc
        gc.collect()
    LAST_EXEC_NS = int((time.perf_counter() - t_all0) * 1e9)
    return out
